# revision 12
# baseline (speedup 1.0000x reference)
"""Trainium2 Bass kernel for nn_AnalysisBand (topk_masking).

Strategy (8 NeuronCores, SPMD):
  - core q handles example b = q//2, time half hf = q%2 (16384 samples + halo).
  - full forward (filterbank conv, example-norm, pos-enc reduce conv, 6-block
    dilated residual stack, to_atoms conv + relu) runs on-device in fp32.
  - example-norm statistics are all-reduced over the 2-core pair with one
    tiny collective.
  - top-k: per (atom, 512-sample) block the device tracks top-8 values +
    indices (DVE max8/max_index), spills the full relu'd activation map to
    device DRAM, and gathers the per-partition top-8-by-v8 blocks (a provable
    superset of any block that can hold >8 of the example's top-64).
  - host merges the per-core block-top-8 lists exactly (threshold t8 = 64th
    largest of the union; blocks with v8 >= t8 are read from the gathered
    full blocks), reconstructs the exact global top-64 (values+positions),
    and scatters the 64 scaled atoms into the output (the final conv of a
    64-nonzero sparse tensor).

Self-contained: only needs the installed concourse/bass toolchain.
"""

import sys
from contextlib import ExitStack

import numpy as np

if "/opt/trn_rl_repo" not in sys.path:
    sys.path.insert(0, "/opt/trn_rl_repo")

import concourse.bass as bass
import concourse.tile as tile
from concourse import bacc, mybir
from concourse import bass_isa
from concourse.bass_utils import run_bass_kernel_spmd

# ---------------- problem constants (hardcoded) ----------------
B = 4
N = 32768
C = 128
KS = 512            # filterbank kernel size
NA = 512            # n atoms
ASZ = 512           # atom size
KSP = 64            # k sparse
DILS = (1, 3, 9, 27, 81, 1)

N_CORES = 8
HALF = N // 2       # 16384 per core
TW = 512            # time tile width
NWT = 33            # workspace tiles (16896 cols)
NET = 32            # exclusive-region tiles
GUARD = 128         # zero guard columns each side
MARG = 256          # halo margin each side
WS = NWT * TW       # 16896 workspace columns
BW = GUARD + WS + GUARD   # 17152 buffer columns
EXC0 = GUARD + MARG       # buffer col of exclusive-region start (384)
XPAD_LEN = 17408          # x window per core: [v0-512, v0+16896)

NBLK = 4 * NET            # 128 (atom-chunk, tile) blocks per partition row
F32 = mybir.dt.float32
U32 = mybir.dt.uint32

_PAIR_GROUPS = [[0, 1], [2, 3], [4, 5], [6, 7]]

_CACHE = {}


class _StopBuild(Exception):
    pass


def _build_program():
    import os
    KN = lambda name: os.environ.get(name, "") == "1"
    STOP = int(os.environ.get("K_STOP", "9"))
    nc = bacc.Bacc("TRN2", target_bir_lowering=False, debug=False,
                   num_devices=N_CORES)

    # ---- per-core external inputs ----
    xpad = nc.dram_tensor("xpad", [1, XPAD_LEN], F32, kind="ExternalInput")
    posb = nc.dram_tensor("posb", [33, BW], F32, kind="ExternalInput")
    bankT = nc.dram_tensor("bankT", [128, 512], F32, kind="ExternalInput")
    redTh = nc.dram_tensor("redTh", [128, 128], F32, kind="ExternalInput")
    redTp = nc.dram_tensor("redTp", [33, 128], F32, kind="ExternalInput")
    dilT = nc.dram_tensor("dilT", [128, 18 * 128], F32, kind="ExternalInput")
    convT = nc.dram_tensor("convT", [128, 6 * 128], F32, kind="ExternalInput")
    atomT = nc.dram_tensor("atomT", [128, 512], F32, kind="ExternalInput")
    mask0 = nc.dram_tensor("mask0", [128, TW], F32, kind="ExternalInput")
    mask32 = nc.dram_tensor("mask32", [128, TW], F32, kind="ExternalInput")

    # ---- outputs ----
    v8s_o = nc.dram_tensor("v8s", [128, NBLK * 8], F32, kind="ExternalOutput")
    i8s_o = nc.dram_tensor("i8s", [128, NBLK * 8], U32, kind="ExternalOutput")
    sblk_o = nc.dram_tensor("sblk", [128, 8], U32, kind="ExternalOutput")
    sdat_o = nc.dram_tensor("sdat", [128, 8 * TW], F32, kind="ExternalOutput")
    ctx_o = nc.dram_tensor("ctxo", [128, 1], F32, kind="ExternalOutput")
    stat_o = nc.dram_tensor("stato", [1, 4], F32, kind="ExternalOutput")

    # ---- internal DRAM ----
    a_dram = nc.dram_tensor("a_dram", [NBLK * 128, TW], F32)   # [bi*128+p, 512]
    st_in = nc.dram_tensor("st_in", [1, 2], F32)
    st_out = nc.dram_tensor("st_out", [1, 2], F32)

    Af = mybir.ActivationFunctionType

    with tile.TileContext(nc) as tc:
      try:
        with ExitStack() as ctx:
            wpool = ctx.enter_context(tc.tile_pool(name="w", bufs=1))
            hpoolA = ctx.enter_context(tc.tile_pool(name="hA", bufs=1))
            hpoolB = ctx.enter_context(tc.tile_pool(name="hB", bufs=1))
            impool = ctx.enter_context(tc.tile_pool(name="im", bufs=2))
            pspool = ctx.enter_context(
                tc.tile_pool(name="ps", bufs=2, space="PSUM"))
            tpool = ctx.enter_context(tc.tile_pool(name="tp", bufs=2))
            apool = ctx.enter_context(tc.tile_pool(name="ap", bufs=3))
            spool = ctx.enter_context(tc.tile_pool(name="st", bufs=1))
            pppool = ctx.enter_context(tc.tile_pool(name="pp", bufs=3))
            gpool = ctx.enter_context(tc.tile_pool(name="gp", bufs=2))

            # ---- weight loads ----
            bank_t = wpool.tile([128, 512], F32)
            nc.sync.dma_start(bank_t[:], bankT.ap())
            redh_t = wpool.tile([128, 128], F32)
            nc.sync.dma_start(redh_t[:], redTh.ap())
            redp_t = wpool.tile([33, 128], F32)
            nc.sync.dma_start(redp_t[:], redTp.ap())
            dil_t = wpool.tile([128, 18 * 128], F32)
            nc.sync.dma_start(dil_t[:], dilT.ap())
            conv_t = wpool.tile([128, 6 * 128], F32)
            nc.sync.dma_start(conv_t[:], convT.ap())
            atom_t = wpool.tile([128, 512], F32)
            nc.sync.dma_start(atom_t[:], atomT.ap())
            m0_t = wpool.tile([128, TW], F32)
            nc.sync.dma_start(m0_t[:], mask0.ap())
            m32_t = wpool.tile([128, TW], F32)
            nc.sync.dma_start(m32_t[:], mask32.ap())

            hA = hpoolA.tile([128, BW], F32)
            hB = hpoolB.tile([128, BW], F32)
            # zero guards (never written again)
            for buf in (hA, hB):
                nc.vector.memset(buf[:, 0:GUARD], 0.0)
                nc.vector.memset(buf[:, GUARD + WS:BW], 0.0)

            # stats accumulators
            ssum = spool.tile([128, NWT], F32)
            ssq = spool.tile([128, NWT], F32)

            # ---------- P1: filterbank conv + stats ----------
            for k in range(NWT):
                xim = impool.tile([128, 896], F32)
                src = bass.AP(xpad, TW * k, [[1, 128], [1, 896]])
                nc.sync.dma_start(xim[:], src)
                ps = pspool.tile([128, TW], F32)
                for kc in range(4):
                    nc.tensor.matmul(
                        ps[:], bank_t[:, kc * 128:(kc + 1) * 128],
                        xim[:, kc * 128:kc * 128 + TW],
                        start=(kc == 0), stop=(kc == 3))
                dst = hA[:, GUARD + k * TW: GUARD + (k + 1) * TW]
                nc.scalar.copy(dst, ps[:])
                # stats over the exclusive region only (read the SBUF copy)
                if k == 0:
                    sub = dst[:, MARG:TW]
                elif k == NWT - 1:
                    sub = dst[:, 0:MARG]
                else:
                    sub = dst[:]
                nc.vector.reduce_sum(ssum[:, k:k + 1], sub, axis=mybir.AxisListType.X)
                scr = tpool.tile([128, TW], F32, tag="z")
                nc.scalar.activation(scr[:, 0:sub.shape[-1]], sub, Af.Square,
                                     accum_out=ssq[:, k:k + 1])

            if STOP < 2:
                raise _StopBuild(None)
            # ---------- stats all-reduce over the pair ----------
            s1 = spool.tile([128, 1], F32)
            s2 = spool.tile([128, 1], F32)
            nc.vector.reduce_sum(s1[:], ssum[:], axis=mybir.AxisListType.X)
            nc.vector.reduce_sum(s2[:], ssq[:], axis=mybir.AxisListType.X)
            ones128 = spool.tile([128, 1], F32)
            nc.vector.memset(ones128[:], 1.0)
            ones1w = spool.tile([1, 128], F32)
            nc.vector.memset(ones1w[:], 1.0)
            stin = spool.tile([1, 2], F32)
            psA = pspool.tile([1, 1], F32, tag="ps2")
            nc.tensor.matmul(psA[:], s1[:], ones128[:], start=True, stop=True)
            nc.vector.tensor_copy(stin[0:1, 0:1], psA[:])
            psB = pspool.tile([1, 1], F32, tag="ps2")
            nc.tensor.matmul(psB[:], s2[:], ones128[:], start=True, stop=True)
            nc.vector.tensor_copy(stin[0:1, 1:2], psB[:])
            nc.sync.dma_start(st_in.ap(), stin[:])
            stout = spool.tile([1, 2], F32)
            if KN("K_SKIP_COLL"):
                nc.sync.dma_start(stout[:], st_in.ap())
                nc.vector.tensor_scalar_mul(stout[:], stout[:], 2.0)
            else:
                nc.gpsimd.collective_compute(
                    "AllReduce", mybir.AluOpType.add,
                    replica_groups=_PAIR_GROUPS,
                    ins=[st_in.ap()], outs=[st_out.ap()])
                nc.sync.dma_start(stout[:], st_out.ap())
            # mu = S/(C*N); m2 = Q/(C*N); var = m2 - mu^2
            invN = 1.0 / (C * N)
            mu = spool.tile([1, 1], F32)
            nc.scalar.mul(mu[:], stout[0:1, 0:1], invN)
            m2 = spool.tile([1, 1], F32)
            nc.scalar.mul(m2[:], stout[0:1, 1:2], invN)
            musq = spool.tile([1, 1], F32)
            nc.vector.tensor_tensor(out=musq[:], in0=mu[:], in1=mu[:],
                                    op=mybir.AluOpType.mult)
            var = spool.tile([1, 1], F32)
            nc.vector.tensor_tensor(out=var[:], in0=m2[:], in1=musq[:],
                                    op=mybir.AluOpType.subtract)
            sig = spool.tile([1, 1], F32)
            nc.scalar.sqrt(sig[:], var[:])
            sige = spool.tile([1, 1], F32)
            nc.vector.tensor_scalar_add(sige[:], sig[:], 1e-8)
            inv = spool.tile([1, 1], F32)
            nc.vector.reciprocal(inv[:], sige[:])
            nbias = spool.tile([1, 1], F32)
            nc.vector.tensor_tensor(out=nbias[:], in0=mu[:], in1=inv[:],
                                    op=mybir.AluOpType.mult)
            nc.scalar.mul(nbias[:], nbias[:], -1.0)
            invb = spool.tile([128, 1], F32)
            psC = pspool.tile([128, 1], F32, tag="ps2")
            nc.tensor.matmul(psC[:], ones1w[:], inv[:], start=True, stop=True)
            nc.vector.tensor_copy(invb[:], psC[:])
            nbb = spool.tile([128, 1], F32)
            psD = pspool.tile([128, 1], F32, tag="ps2")
            nc.tensor.matmul(psD[:], ones1w[:], nbias[:], start=True, stop=True)
            nc.vector.tensor_copy(nbb[:], psD[:])
            stdbg = spool.tile([1, 4], F32)
            nc.vector.tensor_copy(stdbg[0:1, 0:1], mu[:])
            nc.vector.tensor_copy(stdbg[0:1, 1:2], sig[:])
            nc.vector.tensor_copy(stdbg[0:1, 2:3], stout[0:1, 0:1])
            nc.vector.tensor_copy(stdbg[0:1, 3:4], stout[0:1, 1:2])
            nc.sync.dma_start(stat_o.ap(), stdbg[:])

            # normalize whole workspace in place (margins get garbage bias,
            # cleaned by the tile-0/32 masks in P2)
            nc.scalar.activation(hA[:, GUARD:GUARD + WS],
                                 hA[:, GUARD:GUARD + WS],
                                 Af.Identity, bias=nbb[:], scale=invb[:])

            if STOP < 3:
                raise _StopBuild(None)
            # ---------- P2: reduce conv (161 -> 128) ----------
            for k in range(NWT):
                pp = pppool.tile([33, TW], F32)
                nc.sync.dma_start(
                    pp[:], posb.ap()[:, GUARD + k * TW:GUARD + (k + 1) * TW])
                ps = pspool.tile([128, TW], F32)
                nc.tensor.matmul(ps[:], redh_t[:],
                                 hA[:, GUARD + k * TW:GUARD + (k + 1) * TW],
                                 start=True, stop=False)
                nc.tensor.matmul(ps[:], redp_t[:], pp[:],
                                 start=False, stop=True)
                dst = hB[:, GUARD + k * TW:GUARD + (k + 1) * TW]
                if k == 0:
                    nc.vector.tensor_tensor(out=dst, in0=ps[:], in1=m0_t[:],
                                            op=mybir.AluOpType.mult)
                elif k == NWT - 1:
                    nc.vector.tensor_tensor(out=dst, in0=ps[:], in1=m32_t[:],
                                            op=mybir.AluOpType.mult)
                else:
                    nc.scalar.copy(dst, ps[:])

            if STOP < 4:
                raise _StopBuild(None)
            # ---------- P3: dilated residual stack ----------
            bufs = (hB, hA)
            for i, d in enumerate(DILS):
                IN = bufs[i % 2]
                OUT = bufs[(i + 1) % 2]
                for k in range(NWT):
                    c0 = GUARD + k * TW
                    ps1 = pspool.tile([128, TW], F32, tag="ps1")
                    for tap in range(3):
                        off = (tap - 1) * d
                        nc.tensor.matmul(
                            ps1[:], dil_t[:, (i * 3 + tap) * 128:(i * 3 + tap + 1) * 128],
                            IN[:, c0 + off:c0 + off + TW],
                            start=(tap == 0), stop=(tap == 2))
                    z = tpool.tile([128, TW], F32, tag="z")
                    if k == 0:
                        nc.vector.tensor_tensor(out=z[:], in0=ps1[:], in1=m0_t[:],
                                                op=mybir.AluOpType.mult)
                    elif k == NWT - 1:
                        nc.vector.tensor_tensor(out=z[:], in0=ps1[:], in1=m32_t[:],
                                                op=mybir.AluOpType.mult)
                    else:
                        nc.scalar.copy(z[:], ps1[:])
                    ps2 = pspool.tile([128, TW], F32, tag="ps2")
                    nc.tensor.matmul(ps2[:], conv_t[:, i * 128:(i + 1) * 128],
                                     z[:], start=True, stop=True)
                    u = tpool.tile([128, TW], F32, tag="u")
                    nc.vector.tensor_tensor(out=u[:], in0=ps2[:],
                                            in1=IN[:, c0:c0 + TW],
                                            op=mybir.AluOpType.add)
                    ul = tpool.tile([128, TW], F32, tag="ul")
                    nc.scalar.mul(ul[:], u[:], 0.2)
                    nc.vector.tensor_tensor(out=OUT[:, c0:c0 + TW],
                                            in0=u[:], in1=ul[:],
                                            op=mybir.AluOpType.max)

            FEAT = bufs[0]  # 6 blocks: ends back in hB

            if STOP < 5:
                raise _StopBuild(None)
            # ---------- P4: context ----------
            ctx_t = spool.tile([128, 1], F32)
            nc.vector.reduce_max(ctx_t[:], FEAT[:, EXC0:EXC0 + NET * TW],
                                 axis=mybir.AxisListType.X)
            nc.sync.dma_start(ctx_o.ap(), ctx_t[:])

            if STOP < 6:
                raise _StopBuild(None)
            # ---------- P5: to_atoms + relu + block top-8 + spill ----------
            v8s_t = spool.tile([128, NBLK * 8], F32)
            i8s_t = spool.tile([128, NBLK * 8], U32)
            for k in range(NET):
                c0 = EXC0 + k * TW
                for ac in range(4):
                    ps = pspool.tile([128, TW], F32, tag="ps")
                    nc.tensor.matmul(ps[:],
                                     atom_t[:, ac * 128:(ac + 1) * 128],
                                     FEAT[:, c0:c0 + TW],
                                     start=True, stop=True)
                    at = apool.tile([128, TW], F32)
                    nc.scalar.activation(at[:], ps[:], Af.Relu)
                    bi = ac * NET + k
                    nc.vector.max(v8s_t[:, bi * 8:(bi + 1) * 8], at[:])
                    nc.vector.max_index(i8s_t[:, bi * 8:(bi + 1) * 8],
                                        v8s_t[:, bi * 8:(bi + 1) * 8], at[:])
                    if not KN("K_SKIP_SPILL"):
                        nc.sync.dma_start(
                            a_dram.ap()[bi * 128:(bi + 1) * 128, :], at[:])
            nc.sync.dma_start(v8s_o.ap(), v8s_t[:])
            nc.sync.dma_start(i8s_o.ap(), i8s_t[:])

            if STOP < 7:
                raise _StopBuild(None)
            # ---------- P6: gather per-partition top-8-by-v8 blocks ----------
            v8col = bass.AP(v8s_t.tensor, v8s_t.offset + 7,
                            [v8s_t.ap[0], [8, NBLK]])
            s8 = spool.tile([128, 8], F32)
            nc.vector.max(s8[:], v8col)
            si8 = spool.tile([128, 8], U32)
            nc.vector.max_index(si8[:], s8[:], v8col)
            nc.sync.dma_start(sblk_o.ap(), si8[:])
            pio = spool.tile([128, 1], U32)
            nc.gpsimd.iota(pio[:], pattern=[[1, 1]], base=0,
                           channel_multiplier=1)
            shl = spool.tile([128, 8], U32)
            nc.vector.tensor_scalar(shl[:], si8[:], 7, None,
                                    op0=mybir.AluOpType.logical_shift_left)
            rowi = spool.tile([128, 8], U32)
            nc.vector.tensor_tensor(out=rowi[:], in0=shl[:],
                                    in1=pio[:].to_broadcast([128, 8]),
                                    op=mybir.AluOpType.add)
            for s in range(8):
                gt = gpool.tile([128, TW], F32)
                if KN("K_SKIP_GATHER") or KN("K_SKIP_SPILL"):
                    nc.vector.memset(gt[:], 0.0)
                else:
                    nc.gpsimd.indirect_dma_start(
                        out=gt[:], out_offset=None,
                        in_=a_dram.ap(),
                        in_offset=bass.IndirectOffsetOnAxis(ap=rowi[:, s:s + 1],
                                                            axis=0))
                nc.sync.dma_start(sdat_o.ap()[:, s * TW:(s + 1) * TW], gt[:])

      except _StopBuild:
        pass
    nc.compile()
    return nc


def _pos_features():
    """Bit-exact match of the reference's jax positional encoding.

    The reference runs on XLA-CPU, whose f32 sin/cos at large arguments
    (up to 2^15) differs from correctly-rounded sin by up to ~8e-4 — enough
    to flip the top-64 selection. Replicate it exactly with jax-on-CPU.
    """
    try:
        import jax
        import jax.numpy as jnp
        with jax.default_device(jax.devices("cpu")[0]):
            p = jnp.linspace(-1.0, 1.0, N)
            feats = [p] + [f((2.0 ** i) * p) for i in range(16)
                           for f in (jnp.sin, jnp.cos)]
            return np.asarray(jnp.stack(feats, 0), np.float32)
    except Exception:
        p = np.linspace(-1.0, 1.0, N, dtype=np.float32)
        feats = [p]
        for i in range(16):
            arg = (np.float32(2.0 ** i) * p).astype(np.float64)
            feats.append(np.sin(arg).astype(np.float32))
            feats.append(np.cos(arg).astype(np.float32))
        return np.stack(feats, 0)  # [33, N]


def kernel(x, bank, atoms, reduce_w, to_atoms_w, dilated_ws, conv_ws,
           _return_results=False, _trace=False):
    x = np.asarray(x, np.float32)
    bank = np.asarray(bank, np.float32)
    atoms = np.asarray(atoms, np.float32)
    reduce_w = np.asarray(reduce_w, np.float32)
    to_atoms_w = np.asarray(to_atoms_w, np.float32)
    dilated_ws = np.asarray(dilated_ws, np.float32)
    conv_ws = np.asarray(conv_ws, np.float32)

    if "prog" not in _CACHE:
        _CACHE["prog"] = _build_program()
    nc = _CACHE["prog"]

    # ---- host-side weight layouts ----
    bankT = np.ascontiguousarray(bank[:, 0, :].T)                  # [512k,128c] -> [128p? ...]
    # bankT tile layout [128, 512]: [p, kc*128 + c] = bank[c, kc*128+p]
    bankT_sb = np.zeros((128, 512), np.float32)
    for kc in range(4):
        bankT_sb[:, kc * 128:(kc + 1) * 128] = bank[:, 0, kc * 128:(kc + 1) * 128].T
    redTh = np.ascontiguousarray(reduce_w[:, :128, 0].T)           # [128ci,128co]
    redTp = np.ascontiguousarray(reduce_w[:, 128:, 0].T)           # [33, 128]
    dilT_sb = np.zeros((128, 18 * 128), np.float32)
    for i in range(6):
        for tap in range(3):
            dilT_sb[:, (i * 3 + tap) * 128:(i * 3 + tap + 1) * 128] = \
                dilated_ws[i, :, :, tap].T
    convT_sb = np.zeros((128, 6 * 128), np.float32)
    for i in range(6):
        convT_sb[:, i * 128:(i + 1) * 128] = conv_ws[i, :, :, 0].T
    atomT_sb = np.ascontiguousarray(to_atoms_w[:, :, 0].T)         # [128ci, 512ao]

    pos = _pos_features()                                          # [33, N]

    in_maps = []
    for q in range(N_CORES):
        b, hf = divmod(q, 2)
        t0 = hf * HALF
        v0 = t0 - MARG
        g0 = v0 - GUARD
        # xpad[i] = x[b, v0-512+i], zero outside
        xb = v0 - KS
        xp = np.zeros((1, XPAD_LEN), np.float32)
        lo, hi = max(0, xb), min(N, xb + XPAD_LEN)
        if hi > lo:
            xp[0, lo - xb:hi - xb] = x[b, lo:hi]
        # posb aligned to buffer cols: col c -> global g0 + c
        pb = np.zeros((33, BW), np.float32)
        lo, hi = max(0, g0), min(N, g0 + BW)
        if hi > lo:
            pb[:, lo - g0:hi - g0] = pos[:, lo:hi]
        # masks for tiles 0 and 32 (buffer cols [128,640) and [16512,17024))
        m0 = np.zeros((128, TW), np.float32)
        m32 = np.zeros((128, TW), np.float32)
        for j in range(TW):
            g_t0 = g0 + GUARD + j
            g_t32 = g0 + GUARD + (NWT - 1) * TW + j
            m0[:, j] = 1.0 if 0 <= g_t0 < N else 0.0
            m32[:, j] = 1.0 if 0 <= g_t32 < N else 0.0
        in_maps.append(dict(
            xpad=xp, posb=pb, bankT=bankT_sb, redTh=redTh, redTp=redTp,
            dilT=dilT_sb, convT=convT_sb, atomT=atomT_sb,
            mask0=m0, mask32=m32))

    res = run_bass_kernel_spmd(nc, in_maps, list(range(N_CORES)),
                               trace=_trace)
    results = res.results

    # ---- host merge ----
    out = np.zeros((B, 1, N + 2 * ASZ - 1), np.float32)
    context = np.zeros((B, C), np.float32)
    n = N
    for b in range(B):
        cand_vals = []
        cand_flat = []
        for hf in range(2):
            r = results[2 * b + hf]
            v8 = r["v8s"].reshape(128, NBLK, 8)
            i8 = r["i8s"].reshape(128, NBLK, 8).astype(np.int64)
            r["_v8"] = v8
            r["_i8"] = i8
        # t8 = 64th largest of the union of per-block top-8s
        allv = np.concatenate([results[2 * b + hf]["_v8"].ravel()
                               for hf in range(2)])
        t8 = np.partition(allv, -KSP)[-KSP]
        for hf in range(2):
            r = results[2 * b + hf]
            v8, i8 = r["_v8"], r["_i8"]
            sblk = r["sblk"].astype(np.int64)            # [128, 8]
            sdat = r["sdat"].reshape(128, 8, TW)
            susp = v8[:, :, 7] >= t8                     # [128, NBLK]
            ps, bis = np.nonzero(susp)
            # every suspicious block must be among the gathered ones
            gathered = {(p, sblk[p, s]) for p in range(128) for s in range(8)}
            for p, bi in zip(ps.tolist(), bis.tolist()):
                assert (p, bi) in gathered, "suspicious block not gathered"
            base_t = hf * HALF
            # non-suspicious: top-8 values with positions
            nsp = ~susp
            pp, bb = np.nonzero(nsp)
            if pp.size:
                vals = v8[pp, bb, :].ravel()
                idxs = i8[pp, bb, :].ravel()
                ac = bb // NET
                k = bb % NET
                at = ac * 128 + pp
                tau = base_t + k * TW
                flat = ((at[:, None] * n) + tau[:, None] + idxs.reshape(-1, 8)).ravel()
                keep = vals >= t8
                cand_vals.append(vals[keep])
                cand_flat.append(flat[keep])
            # suspicious: all 512 values from gathered data
            slot_of = {}
            for p in range(128):
                for s in range(8):
                    slot_of[(p, sblk[p, s])] = s
            for p, bi in zip(ps.tolist(), bis.tolist()):
                s = slot_of[(p, bi)]
                vals = sdat[p, s, :]
                ac, k = divmod(bi, NET)
                at = ac * 128 + p
                tau = base_t + k * TW
                flat = at * n + tau + np.arange(TW)
                keep = vals >= t8
                cand_vals.append(vals[keep])
                cand_flat.append(flat[keep])
        cand_vals = np.concatenate(cand_vals)
        cand_flat = np.concatenate(cand_flat)
        # dedupe (a block's top-8 can't double-count: susp excluded; but be safe)
        _, uniq = np.unique(cand_flat, return_index=True)
        cand_vals, cand_flat = cand_vals[uniq], cand_flat[uniq]
        # exact top-64, ties by lowest flat index
        order = np.lexsort((cand_flat, -cand_vals))[:KSP]
        for ci in order:
            v = cand_vals[ci]
            at, tau = divmod(int(cand_flat[ci]), n)
            out[b, 0, tau + ASZ:tau + 2 * ASZ] += v * atoms[at]
        context[b] = np.maximum(results[2 * b]["ctxo"][:, 0],
                                results[2 * b + 1]["ctxo"][:, 0])

    if _return_results:
        return (out, context), res
    return out, context


# revision 14
# speedup vs baseline: 1.1654x; 1.1654x over previous
"""Trainium2 Bass kernel for nn_AnalysisBand (topk_masking).

Strategy (8 NeuronCores, SPMD):
  - core q handles example b = q//2, time half hf = q%2 (16384 samples + halo).
  - full forward (filterbank conv, example-norm, pos-enc reduce conv, 6-block
    dilated residual stack, to_atoms conv + relu) runs on-device in fp32.
  - example-norm statistics are all-reduced over the 2-core pair with one
    tiny collective.
  - top-k: per (atom, 512-sample) block the device tracks top-8 values +
    indices (DVE max8/max_index), spills the full relu'd activation map to
    device DRAM, and gathers the per-partition top-8-by-v8 blocks (a provable
    superset of any block that can hold >8 of the example's top-64).
  - host merges the per-core block-top-8 lists exactly (threshold t8 = 64th
    largest of the union; blocks with v8 >= t8 are read from the gathered
    full blocks), reconstructs the exact global top-64 (values+positions),
    and scatters the 64 scaled atoms into the output (the final conv of a
    64-nonzero sparse tensor).

Self-contained: only needs the installed concourse/bass toolchain.
"""

import sys
from contextlib import ExitStack

import numpy as np

if "/opt/trn_rl_repo" not in sys.path:
    sys.path.insert(0, "/opt/trn_rl_repo")

import concourse.bass as bass
import concourse.tile as tile
from concourse import bacc, mybir
from concourse import bass_isa
from concourse.bass_utils import run_bass_kernel_spmd

# ---------------- problem constants (hardcoded) ----------------
B = 4
N = 32768
C = 128
KS = 512            # filterbank kernel size
NA = 512            # n atoms
ASZ = 512           # atom size
KSP = 64            # k sparse
DILS = (1, 3, 9, 27, 81, 1)

N_CORES = 8
HALF = N // 2       # 16384 per core
TW = 512            # time tile width
NWT = 33            # workspace tiles (16896 cols)
NET = 32            # exclusive-region tiles
GUARD = 128         # zero guard columns each side
MARG = 256          # halo margin each side
WS = NWT * TW       # 16896 workspace columns
BW = GUARD + WS + GUARD   # 17152 buffer columns
EXC0 = GUARD + MARG       # buffer col of exclusive-region start (384)
XPAD_LEN = 17408          # x window per core: [v0-512, v0+16896)

NBLK = 4 * NET            # 128 (atom-chunk, tile) blocks per partition row
F32 = mybir.dt.float32
U32 = mybir.dt.uint32

_PAIR_GROUPS = [[0, 1], [2, 3], [4, 5], [6, 7]]

_CACHE = {}


class _StopBuild(Exception):
    pass


def _build_program():
    import os
    KN = lambda name: os.environ.get(name, "") == "1"
    STOP = int(os.environ.get("K_STOP", "9"))
    nc = bacc.Bacc("TRN2", target_bir_lowering=False, debug=False,
                   num_devices=N_CORES)

    # ---- per-core external inputs ----
    xpad = nc.dram_tensor("xpad", [1, XPAD_LEN], F32, kind="ExternalInput")
    posb = nc.dram_tensor("posb", [33, BW], F32, kind="ExternalInput")
    bankT = nc.dram_tensor("bankT", [128, 512], F32, kind="ExternalInput")
    redTh = nc.dram_tensor("redTh", [128, 128], F32, kind="ExternalInput")
    redTp = nc.dram_tensor("redTp", [33, 128], F32, kind="ExternalInput")
    dilT = nc.dram_tensor("dilT", [128, 18 * 128], F32, kind="ExternalInput")
    atomT = nc.dram_tensor("atomT", [128, 512], F32, kind="ExternalInput")
    mask0 = nc.dram_tensor("mask0", [128, TW], F32, kind="ExternalInput")
    mask32 = nc.dram_tensor("mask32", [128, TW], F32, kind="ExternalInput")

    # ---- outputs ----
    v8s_o = nc.dram_tensor("v8s", [128, NBLK * 8], F32, kind="ExternalOutput")
    i8s_o = nc.dram_tensor("i8s", [128, NBLK * 8], U32, kind="ExternalOutput")
    sblk_o = nc.dram_tensor("sblk", [128, 8], U32, kind="ExternalOutput")
    sdat_o = nc.dram_tensor("sdat", [128, 8 * TW], F32, kind="ExternalOutput")
    ctx_o = nc.dram_tensor("ctxo", [128, 1], F32, kind="ExternalOutput")
    stat_o = nc.dram_tensor("stato", [1, 4], F32, kind="ExternalOutput")

    # ---- internal DRAM ----
    a_dram = nc.dram_tensor("a_dram", [NBLK * 128, TW], F32)   # [bi*128+p, 512]
    st_in = nc.dram_tensor("st_in", [1, 2], F32)
    st_out = nc.dram_tensor("st_out", [1, 2], F32)

    Af = mybir.ActivationFunctionType

    with tile.TileContext(nc) as tc:
      try:
        with ExitStack() as ctx:
            wpool = ctx.enter_context(tc.tile_pool(name="w", bufs=1))
            hpoolA = ctx.enter_context(tc.tile_pool(name="hA", bufs=1))
            hpoolB = ctx.enter_context(tc.tile_pool(name="hB", bufs=1))
            impool = ctx.enter_context(tc.tile_pool(name="im", bufs=2))
            pspool = ctx.enter_context(
                tc.tile_pool(name="ps", bufs=3, space="PSUM"))
            tpool = ctx.enter_context(tc.tile_pool(name="tp", bufs=2))
            apool = ctx.enter_context(tc.tile_pool(name="ap", bufs=3))
            spool = ctx.enter_context(tc.tile_pool(name="st", bufs=1))
            pppool = ctx.enter_context(tc.tile_pool(name="pp", bufs=3))
            gpool = ctx.enter_context(tc.tile_pool(name="gp", bufs=2))

            # ---- weight loads ----
            bank_t = wpool.tile([128, 512], F32)
            nc.sync.dma_start(bank_t[:], bankT.ap())
            redh_t = wpool.tile([128, 128], F32)
            nc.sync.dma_start(redh_t[:], redTh.ap())
            redp_t = wpool.tile([33, 128], F32)
            nc.sync.dma_start(redp_t[:], redTp.ap())
            dil_t = wpool.tile([128, 18 * 128], F32)
            nc.sync.dma_start(dil_t[:], dilT.ap())
            atom_t = wpool.tile([128, 512], F32)
            nc.sync.dma_start(atom_t[:], atomT.ap())
            m0_t = wpool.tile([128, TW], F32)
            nc.sync.dma_start(m0_t[:], mask0.ap())
            m32_t = wpool.tile([128, TW], F32)
            nc.sync.dma_start(m32_t[:], mask32.ap())

            hA = hpoolA.tile([128, BW], F32)
            hB = hpoolB.tile([128, BW], F32)
            # zero guards (never written again)
            for buf in (hA, hB):
                nc.vector.memset(buf[:, 0:GUARD], 0.0)
                nc.vector.memset(buf[:, GUARD + WS:BW], 0.0)

            # stats accumulators
            ssum = spool.tile([128, NWT], F32)
            ssq = spool.tile([128, NWT], F32)

            # ---------- P1: filterbank conv + stats ----------
            for k in range(NWT):
                xim = impool.tile([128, 896], F32)
                src = bass.AP(xpad, TW * k, [[1, 128], [1, 896]])
                nc.sync.dma_start(xim[:], src)
                ps = pspool.tile([128, TW], F32)
                for kc in range(4):
                    nc.tensor.matmul(
                        ps[:], bank_t[:, kc * 128:(kc + 1) * 128],
                        xim[:, kc * 128:kc * 128 + TW],
                        start=(kc == 0), stop=(kc == 3))
                dst = hA[:, GUARD + k * TW: GUARD + (k + 1) * TW]
                nc.scalar.copy(dst, ps[:])
                # stats over the exclusive region only (read the SBUF copy)
                if k == 0:
                    sub = dst[:, MARG:TW]
                elif k == NWT - 1:
                    sub = dst[:, 0:MARG]
                else:
                    sub = dst[:]
                nc.vector.reduce_sum(ssum[:, k:k + 1], sub, axis=mybir.AxisListType.X)
                scr = tpool.tile([128, TW], F32, tag="z")
                nc.scalar.activation(scr[:, 0:sub.shape[-1]], sub, Af.Square,
                                     accum_out=ssq[:, k:k + 1])

            if STOP < 2:
                raise _StopBuild(None)
            # ---------- stats all-reduce over the pair ----------
            s1 = spool.tile([128, 1], F32)
            s2 = spool.tile([128, 1], F32)
            nc.vector.reduce_sum(s1[:], ssum[:], axis=mybir.AxisListType.X)
            nc.vector.reduce_sum(s2[:], ssq[:], axis=mybir.AxisListType.X)
            ones128 = spool.tile([128, 1], F32)
            nc.vector.memset(ones128[:], 1.0)
            ones1w = spool.tile([1, 128], F32)
            nc.vector.memset(ones1w[:], 1.0)
            stin = spool.tile([1, 2], F32)
            psA = pspool.tile([1, 1], F32, tag="ps2", bufs=1)
            nc.tensor.matmul(psA[:], s1[:], ones128[:], start=True, stop=True)
            nc.vector.tensor_copy(stin[0:1, 0:1], psA[:])
            psB = pspool.tile([1, 1], F32, tag="ps2", bufs=1)
            nc.tensor.matmul(psB[:], s2[:], ones128[:], start=True, stop=True)
            nc.vector.tensor_copy(stin[0:1, 1:2], psB[:])
            nc.sync.dma_start(st_in.ap(), stin[:])
            stout = spool.tile([1, 2], F32)
            if KN("K_SKIP_COLL"):
                nc.sync.dma_start(stout[:], st_in.ap())
                nc.vector.tensor_scalar_mul(stout[:], stout[:], 2.0)
            else:
                nc.gpsimd.collective_compute(
                    "AllReduce", mybir.AluOpType.add,
                    replica_groups=_PAIR_GROUPS,
                    ins=[st_in.ap()], outs=[st_out.ap()])
                nc.sync.dma_start(stout[:], st_out.ap())
            # mu = S/(C*N); m2 = Q/(C*N); var = m2 - mu^2
            invN = 1.0 / (C * N)
            mu = spool.tile([1, 1], F32)
            nc.scalar.mul(mu[:], stout[0:1, 0:1], invN)
            m2 = spool.tile([1, 1], F32)
            nc.scalar.mul(m2[:], stout[0:1, 1:2], invN)
            musq = spool.tile([1, 1], F32)
            nc.vector.tensor_tensor(out=musq[:], in0=mu[:], in1=mu[:],
                                    op=mybir.AluOpType.mult)
            var = spool.tile([1, 1], F32)
            nc.vector.tensor_tensor(out=var[:], in0=m2[:], in1=musq[:],
                                    op=mybir.AluOpType.subtract)
            sig = spool.tile([1, 1], F32)
            nc.scalar.sqrt(sig[:], var[:])
            sige = spool.tile([1, 1], F32)
            nc.vector.tensor_scalar_add(sige[:], sig[:], 1e-8)
            inv = spool.tile([1, 1], F32)
            nc.vector.reciprocal(inv[:], sige[:])
            nbias = spool.tile([1, 1], F32)
            nc.vector.tensor_tensor(out=nbias[:], in0=mu[:], in1=inv[:],
                                    op=mybir.AluOpType.mult)
            nc.scalar.mul(nbias[:], nbias[:], -1.0)
            invb = spool.tile([128, 1], F32)
            psC = pspool.tile([128, 1], F32, tag="ps2", bufs=1)
            nc.tensor.matmul(psC[:], ones1w[:], inv[:], start=True, stop=True)
            nc.vector.tensor_copy(invb[:], psC[:])
            nbb = spool.tile([128, 1], F32)
            psD = pspool.tile([128, 1], F32, tag="ps2", bufs=1)
            nc.tensor.matmul(psD[:], ones1w[:], nbias[:], start=True, stop=True)
            nc.vector.tensor_copy(nbb[:], psD[:])
            stdbg = spool.tile([1, 4], F32)
            nc.vector.tensor_copy(stdbg[0:1, 0:1], mu[:])
            nc.vector.tensor_copy(stdbg[0:1, 1:2], sig[:])
            nc.vector.tensor_copy(stdbg[0:1, 2:3], stout[0:1, 0:1])
            nc.vector.tensor_copy(stdbg[0:1, 3:4], stout[0:1, 1:2])
            nc.sync.dma_start(stat_o.ap(), stdbg[:])

            # normalize whole workspace in place (margins get garbage bias,
            # cleaned by the tile-0/32 masks in P2)
            nc.scalar.activation(hA[:, GUARD:GUARD + WS],
                                 hA[:, GUARD:GUARD + WS],
                                 Af.Identity, bias=nbb[:], scale=invb[:])

            if STOP < 3:
                raise _StopBuild(None)
            # ---------- P2: reduce conv (161 -> 128) ----------
            for k in range(NWT):
                pp = pppool.tile([33, TW], F32)
                nc.sync.dma_start(
                    pp[:], posb.ap()[:, GUARD + k * TW:GUARD + (k + 1) * TW])
                ps = pspool.tile([128, TW], F32)
                nc.tensor.matmul(ps[:], redh_t[:],
                                 hA[:, GUARD + k * TW:GUARD + (k + 1) * TW],
                                 start=True, stop=False)
                nc.tensor.matmul(ps[:], redp_t[:], pp[:],
                                 start=False, stop=True)
                dst = hB[:, GUARD + k * TW:GUARD + (k + 1) * TW]
                if k == 0:
                    nc.vector.tensor_tensor(out=dst, in0=ps[:], in1=m0_t[:],
                                            op=mybir.AluOpType.mult)
                elif k == NWT - 1:
                    nc.vector.tensor_tensor(out=dst, in0=ps[:], in1=m32_t[:],
                                            op=mybir.AluOpType.mult)
                else:
                    nc.scalar.copy(dst, ps[:])

            if STOP < 4:
                raise _StopBuild(None)
            # ---------- P3: dilated residual stack ----------
            bufs = (hB, hA)
            for i, d in enumerate(DILS):
                IN = bufs[i % 2]
                OUT = bufs[(i + 1) % 2]
                for k in range(NWT):
                    c0 = GUARD + k * TW
                    ps1 = pspool.tile([128, TW], F32, tag="ps1")
                    for tap in range(3):
                        off = (tap - 1) * d
                        nc.tensor.matmul(
                            ps1[:], dil_t[:, (i * 3 + tap) * 128:(i * 3 + tap + 1) * 128],
                            IN[:, c0 + off:c0 + off + TW],
                            start=(tap == 0), stop=(tap == 2))
                    # u = conv_out (masked at the example edge) + residual
                    u = tpool.tile([128, TW], F32, tag="u")
                    if k == 0 or k == NWT - 1:
                        mt = m0_t if k == 0 else m32_t
                        z = tpool.tile([128, TW], F32, tag="z")
                        nc.vector.tensor_tensor(out=z[:], in0=ps1[:], in1=mt[:],
                                                op=mybir.AluOpType.mult)
                        nc.vector.tensor_tensor(out=u[:], in0=z[:],
                                                in1=IN[:, c0:c0 + TW],
                                                op=mybir.AluOpType.add)
                    else:
                        nc.vector.tensor_tensor(out=u[:], in0=ps1[:],
                                                in1=IN[:, c0:c0 + TW],
                                                op=mybir.AluOpType.add)
                    ul = tpool.tile([128, TW], F32, tag="ul")
                    nc.scalar.mul(ul[:], u[:], 0.2)
                    nc.vector.tensor_tensor(out=OUT[:, c0:c0 + TW],
                                            in0=u[:], in1=ul[:],
                                            op=mybir.AluOpType.max)

            FEAT = bufs[0]  # 6 blocks: ends back in hB

            if STOP < 5:
                raise _StopBuild(None)
            # ---------- P4: context ----------
            ctx_t = spool.tile([128, 1], F32)
            nc.vector.reduce_max(ctx_t[:], FEAT[:, EXC0:EXC0 + NET * TW],
                                 axis=mybir.AxisListType.X)
            nc.sync.dma_start(ctx_o.ap(), ctx_t[:])

            if STOP < 6:
                raise _StopBuild(None)
            # ---------- P5: to_atoms + relu + block top-8 + spill ----------
            v8s_t = spool.tile([128, NBLK * 8], F32)
            i8s_t = spool.tile([128, NBLK * 8], U32)
            for k in range(NET):
                c0 = EXC0 + k * TW
                for ac in range(4):
                    ps = pspool.tile([128, TW], F32, tag="ps")
                    nc.tensor.matmul(ps[:],
                                     atom_t[:, ac * 128:(ac + 1) * 128],
                                     FEAT[:, c0:c0 + TW],
                                     start=True, stop=True)
                    at = apool.tile([128, TW], F32)
                    nc.scalar.activation(at[:], ps[:], Af.Relu)
                    bi = ac * NET + k
                    nc.vector.max(v8s_t[:, bi * 8:(bi + 1) * 8], at[:])
                    nc.vector.max_index(i8s_t[:, bi * 8:(bi + 1) * 8],
                                        v8s_t[:, bi * 8:(bi + 1) * 8], at[:])
                    if not KN("K_SKIP_SPILL"):
                        nc.sync.dma_start(
                            a_dram.ap()[bi * 128:(bi + 1) * 128, :], at[:])
            nc.sync.dma_start(v8s_o.ap(), v8s_t[:])
            nc.sync.dma_start(i8s_o.ap(), i8s_t[:])

            if STOP < 7:
                raise _StopBuild(None)
            # ---------- P6: gather per-partition top-8-by-v8 blocks ----------
            v8col = bass.AP(v8s_t.tensor, v8s_t.offset + 7,
                            [v8s_t.ap[0], [8, NBLK]])
            s8 = spool.tile([128, 8], F32)
            nc.vector.max(s8[:], v8col)
            si8 = spool.tile([128, 8], U32)
            nc.vector.max_index(si8[:], s8[:], v8col)
            nc.sync.dma_start(sblk_o.ap(), si8[:])
            pio = spool.tile([128, 1], U32)
            nc.gpsimd.iota(pio[:], pattern=[[1, 1]], base=0,
                           channel_multiplier=1)
            shl = spool.tile([128, 8], U32)
            nc.vector.tensor_scalar(shl[:], si8[:], 7, None,
                                    op0=mybir.AluOpType.logical_shift_left)
            rowi = spool.tile([128, 8], U32)
            nc.vector.tensor_tensor(out=rowi[:], in0=shl[:],
                                    in1=pio[:].to_broadcast([128, 8]),
                                    op=mybir.AluOpType.add)
            for s in range(8):
                gt = gpool.tile([128, TW], F32)
                if KN("K_SKIP_GATHER") or KN("K_SKIP_SPILL"):
                    nc.vector.memset(gt[:], 0.0)
                else:
                    nc.gpsimd.indirect_dma_start(
                        out=gt[:], out_offset=None,
                        in_=a_dram.ap(),
                        in_offset=bass.IndirectOffsetOnAxis(ap=rowi[:, s:s + 1],
                                                            axis=0))
                nc.sync.dma_start(sdat_o.ap()[:, s * TW:(s + 1) * TW], gt[:])

      except _StopBuild:
        pass
    nc.compile()
    return nc


def _pos_features():
    """Bit-exact match of the reference's jax positional encoding.

    The reference runs on XLA-CPU, whose f32 sin/cos at large arguments
    (up to 2^15) differs from correctly-rounded sin by up to ~8e-4 — enough
    to flip the top-64 selection. Replicate it exactly with jax-on-CPU.
    """
    try:
        import jax
        import jax.numpy as jnp
        with jax.default_device(jax.devices("cpu")[0]):
            p = jnp.linspace(-1.0, 1.0, N)
            feats = [p] + [f((2.0 ** i) * p) for i in range(16)
                           for f in (jnp.sin, jnp.cos)]
            return np.asarray(jnp.stack(feats, 0), np.float32)
    except Exception:
        p = np.linspace(-1.0, 1.0, N, dtype=np.float32)
        feats = [p]
        for i in range(16):
            arg = (np.float32(2.0 ** i) * p).astype(np.float64)
            feats.append(np.sin(arg).astype(np.float32))
            feats.append(np.cos(arg).astype(np.float32))
        return np.stack(feats, 0)  # [33, N]


def kernel(x, bank, atoms, reduce_w, to_atoms_w, dilated_ws, conv_ws,
           _return_results=False, _trace=False):
    x = np.asarray(x, np.float32)
    bank = np.asarray(bank, np.float32)
    atoms = np.asarray(atoms, np.float32)
    reduce_w = np.asarray(reduce_w, np.float32)
    to_atoms_w = np.asarray(to_atoms_w, np.float32)
    dilated_ws = np.asarray(dilated_ws, np.float32)
    conv_ws = np.asarray(conv_ws, np.float32)

    if "prog" not in _CACHE:
        _CACHE["prog"] = _build_program()
    nc = _CACHE["prog"]

    # ---- host-side weight layouts ----
    bankT = np.ascontiguousarray(bank[:, 0, :].T)                  # [512k,128c] -> [128p? ...]
    # bankT tile layout [128, 512]: [p, kc*128 + c] = bank[c, kc*128+p]
    bankT_sb = np.zeros((128, 512), np.float32)
    for kc in range(4):
        bankT_sb[:, kc * 128:(kc + 1) * 128] = bank[:, 0, kc * 128:(kc + 1) * 128].T
    redTh = np.ascontiguousarray(reduce_w[:, :128, 0].T)           # [128ci,128co]
    redTp = np.ascontiguousarray(reduce_w[:, 128:, 0].T)           # [33, 128]
    dilT_sb = np.zeros((128, 18 * 128), np.float32)
    for i in range(6):
        w2 = conv_ws[i, :, :, 0].astype(np.float64)       # [co, c2]
        for tap in range(3):
            comp = w2 @ dilated_ws[i, :, :, tap].astype(np.float64)  # [co, ci]
            dilT_sb[:, (i * 3 + tap) * 128:(i * 3 + tap + 1) * 128] = \
                comp.T.astype(np.float32)
    atomT_sb = np.ascontiguousarray(to_atoms_w[:, :, 0].T)         # [128ci, 512ao]

    pos = _pos_features()                                          # [33, N]

    in_maps = []
    for q in range(N_CORES):
        b, hf = divmod(q, 2)
        t0 = hf * HALF
        v0 = t0 - MARG
        g0 = v0 - GUARD
        # xpad[i] = x[b, v0-512+i], zero outside
        xb = v0 - KS
        xp = np.zeros((1, XPAD_LEN), np.float32)
        lo, hi = max(0, xb), min(N, xb + XPAD_LEN)
        if hi > lo:
            xp[0, lo - xb:hi - xb] = x[b, lo:hi]
        # posb aligned to buffer cols: col c -> global g0 + c
        pb = np.zeros((33, BW), np.float32)
        lo, hi = max(0, g0), min(N, g0 + BW)
        if hi > lo:
            pb[:, lo - g0:hi - g0] = pos[:, lo:hi]
        # masks for tiles 0 and 32 (buffer cols [128,640) and [16512,17024))
        m0 = np.zeros((128, TW), np.float32)
        m32 = np.zeros((128, TW), np.float32)
        for j in range(TW):
            g_t0 = g0 + GUARD + j
            g_t32 = g0 + GUARD + (NWT - 1) * TW + j
            m0[:, j] = 1.0 if 0 <= g_t0 < N else 0.0
            m32[:, j] = 1.0 if 0 <= g_t32 < N else 0.0
        in_maps.append(dict(
            xpad=xp, posb=pb, bankT=bankT_sb, redTh=redTh, redTp=redTp,
            dilT=dilT_sb, atomT=atomT_sb,
            mask0=m0, mask32=m32))

    res = run_bass_kernel_spmd(nc, in_maps, list(range(N_CORES)),
                               trace=_trace)
    results = res.results

    # ---- host merge ----
    out = np.zeros((B, 1, N + 2 * ASZ - 1), np.float32)
    context = np.zeros((B, C), np.float32)
    n = N
    for b in range(B):
        cand_vals = []
        cand_flat = []
        for hf in range(2):
            r = results[2 * b + hf]
            v8 = r["v8s"].reshape(128, NBLK, 8)
            i8 = r["i8s"].reshape(128, NBLK, 8).astype(np.int64)
            r["_v8"] = v8
            r["_i8"] = i8
        # t8 = 64th largest of the union of per-block top-8s
        allv = np.concatenate([results[2 * b + hf]["_v8"].ravel()
                               for hf in range(2)])
        t8 = np.partition(allv, -KSP)[-KSP]
        for hf in range(2):
            r = results[2 * b + hf]
            v8, i8 = r["_v8"], r["_i8"]
            sblk = r["sblk"].astype(np.int64)            # [128, 8]
            sdat = r["sdat"].reshape(128, 8, TW)
            susp = v8[:, :, 7] >= t8                     # [128, NBLK]
            ps, bis = np.nonzero(susp)
            # every suspicious block must be among the gathered ones
            gathered = {(p, sblk[p, s]) for p in range(128) for s in range(8)}
            for p, bi in zip(ps.tolist(), bis.tolist()):
                assert (p, bi) in gathered, "suspicious block not gathered"
            base_t = hf * HALF
            # non-suspicious: top-8 values with positions
            nsp = ~susp
            pp, bb = np.nonzero(nsp)
            if pp.size:
                vals = v8[pp, bb, :].ravel()
                idxs = i8[pp, bb, :].ravel()
                ac = bb // NET
                k = bb % NET
                at = ac * 128 + pp
                tau = base_t + k * TW
                flat = ((at[:, None] * n) + tau[:, None] + idxs.reshape(-1, 8)).ravel()
                keep = vals >= t8
                cand_vals.append(vals[keep])
                cand_flat.append(flat[keep])
            # suspicious: all 512 values from gathered data
            slot_of = {}
            for p in range(128):
                for s in range(8):
                    slot_of[(p, sblk[p, s])] = s
            for p, bi in zip(ps.tolist(), bis.tolist()):
                s = slot_of[(p, bi)]
                vals = sdat[p, s, :]
                ac, k = divmod(bi, NET)
                at = ac * 128 + p
                tau = base_t + k * TW
                flat = at * n + tau + np.arange(TW)
                keep = vals >= t8
                cand_vals.append(vals[keep])
                cand_flat.append(flat[keep])
        cand_vals = np.concatenate(cand_vals)
        cand_flat = np.concatenate(cand_flat)
        # dedupe (a block's top-8 can't double-count: susp excluded; but be safe)
        _, uniq = np.unique(cand_flat, return_index=True)
        cand_vals, cand_flat = cand_vals[uniq], cand_flat[uniq]
        # exact top-64, ties by lowest flat index
        order = np.lexsort((cand_flat, -cand_vals))[:KSP]
        for ci in order:
            v = cand_vals[ci]
            at, tau = divmod(int(cand_flat[ci]), n)
            out[b, 0, tau + ASZ:tau + 2 * ASZ] += v * atoms[at]
        context[b] = np.maximum(results[2 * b]["ctxo"][:, 0],
                                results[2 * b + 1]["ctxo"][:, 0])

    if _return_results:
        return (out, context), res
    return out, context


# revision 16
# speedup vs baseline: 1.2012x; 1.0307x over previous
"""Trainium2 Bass kernel for nn_AnalysisBand (topk_masking).

Strategy (8 NeuronCores, SPMD):
  - core q handles example b = q//2, time half hf = q%2 (16384 samples + halo).
  - full forward (filterbank conv, example-norm, pos-enc reduce conv, 6-block
    dilated residual stack, to_atoms conv + relu) runs on-device in fp32.
  - example-norm statistics are all-reduced over the 2-core pair with one
    tiny collective.
  - top-k: per (atom, 512-sample) block the device tracks top-8 values +
    indices (DVE max8/max_index), spills the full relu'd activation map to
    device DRAM, and gathers the per-partition top-8-by-v8 blocks (a provable
    superset of any block that can hold >8 of the example's top-64).
  - host merges the per-core block-top-8 lists exactly (threshold t8 = 64th
    largest of the union; blocks with v8 >= t8 are read from the gathered
    full blocks), reconstructs the exact global top-64 (values+positions),
    and scatters the 64 scaled atoms into the output (the final conv of a
    64-nonzero sparse tensor).

Self-contained: only needs the installed concourse/bass toolchain.
"""

import sys
from contextlib import ExitStack

import numpy as np

if "/opt/trn_rl_repo" not in sys.path:
    sys.path.insert(0, "/opt/trn_rl_repo")

import concourse.bass as bass
import concourse.tile as tile
from concourse import bacc, mybir
from concourse import bass_isa
from concourse.bass_utils import run_bass_kernel_spmd

# ---------------- problem constants (hardcoded) ----------------
B = 4
N = 32768
C = 128
KS = 512            # filterbank kernel size
NA = 512            # n atoms
ASZ = 512           # atom size
KSP = 64            # k sparse
DILS = (1, 3, 9, 27, 81, 1)

N_CORES = 8
HALF = N // 2       # 16384 per core
TW = 512            # time tile width
NWT = 33            # workspace tiles (16896 cols)
NET = 32            # exclusive-region tiles
GUARD = 128         # zero guard columns each side
MARG = 256          # halo margin each side
WS = NWT * TW       # 16896 workspace columns
BW = GUARD + WS + GUARD   # 17152 buffer columns
EXC0 = GUARD + MARG       # buffer col of exclusive-region start (384)
XPAD_LEN = 17408          # x window per core: [v0-512, v0+16896)

NBLK = 4 * NET            # 128 (atom-chunk, tile) blocks per partition row
F32 = mybir.dt.float32
U32 = mybir.dt.uint32
BF16 = mybir.dt.bfloat16

_PAIR_GROUPS = [[0, 1], [2, 3], [4, 5], [6, 7]]

_CACHE = {}


class _StopBuild(Exception):
    pass


def _build_program():
    import os
    KN = lambda name: os.environ.get(name, "") == "1"
    STOP = int(os.environ.get("K_STOP", "9"))
    nc = bacc.Bacc("TRN2", target_bir_lowering=False, debug=False,
                   num_devices=N_CORES)

    # ---- per-core external inputs ----
    xpad_h = nc.dram_tensor("xpad_h", [1, XPAD_LEN], BF16, kind="ExternalInput")
    xpad_l = nc.dram_tensor("xpad_l", [1, XPAD_LEN], BF16, kind="ExternalInput")
    posb = nc.dram_tensor("posb", [33, BW], F32, kind="ExternalInput")
    bankT_h = nc.dram_tensor("bankT_h", [128, 512], BF16, kind="ExternalInput")
    bankT_l = nc.dram_tensor("bankT_l", [128, 512], BF16, kind="ExternalInput")
    redTh = nc.dram_tensor("redTh", [128, 128], F32, kind="ExternalInput")
    redTp = nc.dram_tensor("redTp", [33, 128], F32, kind="ExternalInput")
    dilT = nc.dram_tensor("dilT", [128, 18 * 128], F32, kind="ExternalInput")
    atomT = nc.dram_tensor("atomT", [128, 512], F32, kind="ExternalInput")
    mask0 = nc.dram_tensor("mask0", [128, TW], F32, kind="ExternalInput")
    mask32 = nc.dram_tensor("mask32", [128, TW], F32, kind="ExternalInput")

    # ---- outputs ----
    v8s_o = nc.dram_tensor("v8s", [128, NBLK * 8], F32, kind="ExternalOutput")
    i8s_o = nc.dram_tensor("i8s", [128, NBLK * 8], U32, kind="ExternalOutput")
    sblk_o = nc.dram_tensor("sblk", [128, 8], U32, kind="ExternalOutput")
    sdat_o = nc.dram_tensor("sdat", [128, 8 * TW], F32, kind="ExternalOutput")
    ctx_o = nc.dram_tensor("ctxo", [128, 1], F32, kind="ExternalOutput")
    stat_o = nc.dram_tensor("stato", [1, 4], F32, kind="ExternalOutput")

    # ---- internal DRAM ----
    a_dram = nc.dram_tensor("a_dram", [NBLK * 128, TW], F32)   # [bi*128+p, 512]
    st_in = nc.dram_tensor("st_in", [1, 2], F32)
    st_out = nc.dram_tensor("st_out", [1, 2], F32)

    Af = mybir.ActivationFunctionType

    with tile.TileContext(nc) as tc:
      try:
        with ExitStack() as ctx:
            wpool = ctx.enter_context(tc.tile_pool(name="w", bufs=1))
            hpoolA = ctx.enter_context(tc.tile_pool(name="hA", bufs=1))
            hpoolB = ctx.enter_context(tc.tile_pool(name="hB", bufs=1))
            impool = ctx.enter_context(tc.tile_pool(name="im", bufs=2))
            pspool = ctx.enter_context(
                tc.tile_pool(name="ps", bufs=3, space="PSUM"))
            tpool = ctx.enter_context(tc.tile_pool(name="tp", bufs=2))
            apool = ctx.enter_context(tc.tile_pool(name="ap", bufs=3))
            spool = ctx.enter_context(tc.tile_pool(name="st", bufs=1))
            pppool = ctx.enter_context(tc.tile_pool(name="pp", bufs=3))
            gpool = ctx.enter_context(tc.tile_pool(name="gp", bufs=2))

            # ---- weight loads ----
            bank_h = wpool.tile([128, 512], BF16)
            nc.sync.dma_start(bank_h[:], bankT_h.ap())
            bank_l = wpool.tile([128, 512], BF16)
            nc.sync.dma_start(bank_l[:], bankT_l.ap())
            redh_t = wpool.tile([128, 128], F32)
            nc.sync.dma_start(redh_t[:], redTh.ap())
            redp_t = wpool.tile([33, 128], F32)
            nc.sync.dma_start(redp_t[:], redTp.ap())
            hA = hpoolA.tile([128, BW], F32)
            hB = hpoolB.tile([128, BW], F32)
            # zero guards (never written again)
            for buf in (hA, hB):
                nc.vector.memset(buf[:, 0:GUARD], 0.0)
                nc.vector.memset(buf[:, GUARD + WS:BW], 0.0)

            # stats accumulators
            ssum = spool.tile([128, NWT], F32)
            ssq = spool.tile([128, NWT], F32)

            # ---------- P1: filterbank conv + stats ----------
            for k in range(NWT):
                xim_h = impool.tile([128, 896], BF16, tag="xh")
                xim_l = impool.tile([128, 896], BF16, tag="xl")
                if k == 0:
                    for q4 in range(4):
                        nc.sync.dma_start(
                            xim_h[q4 * 32:(q4 + 1) * 32, :],
                            bass.AP(xpad_h, TW * k + q4 * 32, [[1, 32], [1, 896]]))
                        nc.sync.dma_start(
                            xim_l[q4 * 32:(q4 + 1) * 32, :],
                            bass.AP(xpad_l, TW * k + q4 * 32, [[1, 32], [1, 896]]))
                else:
                    nc.sync.dma_start(xim_h[:],
                                      bass.AP(xpad_h, TW * k, [[1, 128], [1, 896]]))
                    nc.sync.dma_start(xim_l[:],
                                      bass.AP(xpad_l, TW * k, [[1, 128], [1, 896]]))
                ps = pspool.tile([128, TW], F32)
                nmm = 12
                imm = 0
                for kc in range(4):
                    for (bw, xw) in ((bank_h, xim_h), (bank_h, xim_l),
                                     (bank_l, xim_h)):
                        nc.tensor.matmul(
                            ps[:], bw[:, kc * 128:(kc + 1) * 128],
                            xw[:, kc * 128:kc * 128 + TW],
                            start=(imm == 0), stop=(imm == nmm - 1))
                        imm += 1
                dst = hA[:, GUARD + k * TW: GUARD + (k + 1) * TW]
                nc.scalar.copy(dst, ps[:])
                # stats over the exclusive region only (read the SBUF copy)
                if k == 0:
                    sub = dst[:, MARG:TW]
                elif k == NWT - 1:
                    sub = dst[:, 0:MARG]
                else:
                    sub = dst[:]
                nc.vector.reduce_sum(ssum[:, k:k + 1], sub, axis=mybir.AxisListType.X)
                scr = tpool.tile([128, TW], F32, tag="z")
                nc.scalar.activation(scr[:, 0:sub.shape[-1]], sub, Af.Square,
                                     accum_out=ssq[:, k:k + 1])

            # late weight loads (needed from P2/P3 on)
            dil_t = wpool.tile([128, 18 * 128], F32)
            nc.sync.dma_start(dil_t[:], dilT.ap())
            atom_t = wpool.tile([128, 512], F32)
            nc.sync.dma_start(atom_t[:], atomT.ap())
            m0_t = wpool.tile([128, TW], F32)
            nc.sync.dma_start(m0_t[:], mask0.ap())
            m32_t = wpool.tile([128, TW], F32)
            nc.sync.dma_start(m32_t[:], mask32.ap())

            if STOP < 2:
                raise _StopBuild(None)
            # ---------- stats all-reduce over the pair ----------
            s1 = spool.tile([128, 1], F32)
            s2 = spool.tile([128, 1], F32)
            nc.vector.reduce_sum(s1[:], ssum[:], axis=mybir.AxisListType.X)
            nc.vector.reduce_sum(s2[:], ssq[:], axis=mybir.AxisListType.X)
            ones128 = spool.tile([128, 1], F32)
            nc.vector.memset(ones128[:], 1.0)
            ones1w = spool.tile([1, 128], F32)
            nc.vector.memset(ones1w[:], 1.0)
            stin = spool.tile([1, 2], F32)
            psA = pspool.tile([1, 1], F32, tag="ps2", bufs=1)
            nc.tensor.matmul(psA[:], s1[:], ones128[:], start=True, stop=True)
            nc.vector.tensor_copy(stin[0:1, 0:1], psA[:])
            psB = pspool.tile([1, 1], F32, tag="ps2", bufs=1)
            nc.tensor.matmul(psB[:], s2[:], ones128[:], start=True, stop=True)
            nc.vector.tensor_copy(stin[0:1, 1:2], psB[:])
            nc.sync.dma_start(st_in.ap(), stin[:])
            stout = spool.tile([1, 2], F32)
            if KN("K_SKIP_COLL"):
                nc.sync.dma_start(stout[:], st_in.ap())
                nc.vector.tensor_scalar_mul(stout[:], stout[:], 2.0)
            else:
                nc.gpsimd.collective_compute(
                    "AllReduce", mybir.AluOpType.add,
                    replica_groups=_PAIR_GROUPS,
                    ins=[st_in.ap()], outs=[st_out.ap()])
                nc.sync.dma_start(stout[:], st_out.ap())
            # mu = S/(C*N); m2 = Q/(C*N); var = m2 - mu^2
            invN = 1.0 / (C * N)
            mu = spool.tile([1, 1], F32)
            nc.scalar.mul(mu[:], stout[0:1, 0:1], invN)
            m2 = spool.tile([1, 1], F32)
            nc.scalar.mul(m2[:], stout[0:1, 1:2], invN)
            musq = spool.tile([1, 1], F32)
            nc.vector.tensor_tensor(out=musq[:], in0=mu[:], in1=mu[:],
                                    op=mybir.AluOpType.mult)
            var = spool.tile([1, 1], F32)
            nc.vector.tensor_tensor(out=var[:], in0=m2[:], in1=musq[:],
                                    op=mybir.AluOpType.subtract)
            sig = spool.tile([1, 1], F32)
            nc.scalar.sqrt(sig[:], var[:])
            sige = spool.tile([1, 1], F32)
            nc.vector.tensor_scalar_add(sige[:], sig[:], 1e-8)
            inv = spool.tile([1, 1], F32)
            nc.vector.reciprocal(inv[:], sige[:])
            nbias = spool.tile([1, 1], F32)
            nc.vector.tensor_tensor(out=nbias[:], in0=mu[:], in1=inv[:],
                                    op=mybir.AluOpType.mult)
            nc.scalar.mul(nbias[:], nbias[:], -1.0)
            invb = spool.tile([128, 1], F32)
            psC = pspool.tile([128, 1], F32, tag="ps2", bufs=1)
            nc.tensor.matmul(psC[:], ones1w[:], inv[:], start=True, stop=True)
            nc.vector.tensor_copy(invb[:], psC[:])
            nbb = spool.tile([128, 1], F32)
            psD = pspool.tile([128, 1], F32, tag="ps2", bufs=1)
            nc.tensor.matmul(psD[:], ones1w[:], nbias[:], start=True, stop=True)
            nc.vector.tensor_copy(nbb[:], psD[:])
            # c1[co] = nbias * sum_ci redh[ci, co]  (row-sums via PE)
            psRS = pspool.tile([128, 1], F32, tag="ps2", bufs=1)
            nc.tensor.matmul(psRS[:], redh_t[:], ones128[:], start=True, stop=True)
            c1b = spool.tile([128, 1], F32)
            nc.scalar.activation(c1b[:], psRS[:], Af.Copy, scale=nbb[:])
            stdbg = spool.tile([1, 4], F32)
            nc.vector.tensor_copy(stdbg[0:1, 0:1], mu[:])
            nc.vector.tensor_copy(stdbg[0:1, 1:2], sig[:])
            nc.vector.tensor_copy(stdbg[0:1, 2:3], stout[0:1, 0:1])
            nc.vector.tensor_copy(stdbg[0:1, 3:4], stout[0:1, 1:2])
            nc.sync.dma_start(stat_o.ap(), stdbg[:])

            if STOP < 3:
                raise _StopBuild(None)
            # ---------- P2: reduce conv (161 -> 128), norm folded ----------
            # y = inv*(Wh @ h_raw) + c1 + Wp @ pos  ==  Wh @ ((h-mu)/sig) + Wp @ pos
            for k in range(NWT):
                pp = pppool.tile([33, TW], F32)
                nc.sync.dma_start(
                    pp[:], posb.ap()[:, GUARD + k * TW:GUARD + (k + 1) * TW])
                ps = pspool.tile([128, TW], F32)
                nc.tensor.matmul(ps[:], redh_t[:],
                                 hA[:, GUARD + k * TW:GUARD + (k + 1) * TW],
                                 start=True, stop=True)
                psP = pspool.tile([128, TW], F32, tag="ps1")
                nc.tensor.matmul(psP[:], redp_t[:], pp[:],
                                 start=True, stop=True)
                t2 = tpool.tile([128, TW], F32, tag="u")
                nc.scalar.activation(t2[:], ps[:], Af.Identity,
                                     bias=c1b[:], scale=invb[:])
                dst = hB[:, GUARD + k * TW:GUARD + (k + 1) * TW]
                if k == 0 or k == NWT - 1:
                    mt = m0_t if k == 0 else m32_t
                    zz = tpool.tile([128, TW], F32, tag="z")
                    nc.vector.tensor_tensor(out=zz[:], in0=t2[:], in1=psP[:],
                                            op=mybir.AluOpType.add)
                    nc.vector.tensor_tensor(out=dst, in0=zz[:], in1=mt[:],
                                            op=mybir.AluOpType.mult)
                else:
                    nc.vector.tensor_tensor(out=dst, in0=t2[:], in1=psP[:],
                                            op=mybir.AluOpType.add)

            if STOP < 4:
                raise _StopBuild(None)
            # ---------- P3: dilated residual stack ----------
            bufs = (hB, hA)
            for i, d in enumerate(DILS):
                IN = bufs[i % 2]
                OUT = bufs[(i + 1) % 2]
                for k in range(NWT):
                    c0 = GUARD + k * TW
                    ps1 = pspool.tile([128, TW], F32, tag="ps1")
                    for tap in range(3):
                        off = (tap - 1) * d
                        nc.tensor.matmul(
                            ps1[:], dil_t[:, (i * 3 + tap) * 128:(i * 3 + tap + 1) * 128],
                            IN[:, c0 + off:c0 + off + TW],
                            start=(tap == 0), stop=(tap == 2))
                    # u = conv_out (masked at the example edge) + residual
                    u = tpool.tile([128, TW], F32, tag="u")
                    if k == 0 or k == NWT - 1:
                        mt = m0_t if k == 0 else m32_t
                        z = tpool.tile([128, TW], F32, tag="z")
                        nc.vector.tensor_tensor(out=z[:], in0=ps1[:], in1=mt[:],
                                                op=mybir.AluOpType.mult)
                        nc.vector.tensor_tensor(out=u[:], in0=z[:],
                                                in1=IN[:, c0:c0 + TW],
                                                op=mybir.AluOpType.add)
                    else:
                        nc.vector.tensor_tensor(out=u[:], in0=ps1[:],
                                                in1=IN[:, c0:c0 + TW],
                                                op=mybir.AluOpType.add)
                    ul = tpool.tile([128, TW], F32, tag="ul")
                    nc.scalar.mul(ul[:], u[:], 0.2)
                    nc.vector.tensor_tensor(out=OUT[:, c0:c0 + TW],
                                            in0=u[:], in1=ul[:],
                                            op=mybir.AluOpType.max)

            FEAT = bufs[0]  # 6 blocks: ends back in hB

            if STOP < 5:
                raise _StopBuild(None)
            # ---------- P4: context ----------
            ctx_t = spool.tile([128, 1], F32)
            nc.vector.reduce_max(ctx_t[:], FEAT[:, EXC0:EXC0 + NET * TW],
                                 axis=mybir.AxisListType.X)
            nc.sync.dma_start(ctx_o.ap(), ctx_t[:])

            if STOP < 6:
                raise _StopBuild(None)
            # ---------- P5: to_atoms + relu + block top-8 + spill ----------
            v8s_t = spool.tile([128, NBLK * 8], F32)
            i8s_t = spool.tile([128, NBLK * 8], U32)
            for k in range(NET):
                c0 = EXC0 + k * TW
                for ac in range(4):
                    ps = pspool.tile([128, TW], F32, tag="ps")
                    nc.tensor.matmul(ps[:],
                                     atom_t[:, ac * 128:(ac + 1) * 128],
                                     FEAT[:, c0:c0 + TW],
                                     start=True, stop=True)
                    at = apool.tile([128, TW], F32)
                    nc.scalar.activation(at[:], ps[:], Af.Relu)
                    bi = ac * NET + k
                    nc.vector.max(v8s_t[:, bi * 8:(bi + 1) * 8], at[:])
                    nc.vector.max_index(i8s_t[:, bi * 8:(bi + 1) * 8],
                                        v8s_t[:, bi * 8:(bi + 1) * 8], at[:])
                    if not KN("K_SKIP_SPILL"):
                        nc.sync.dma_start(
                            a_dram.ap()[bi * 128:(bi + 1) * 128, :], at[:])
            nc.sync.dma_start(v8s_o.ap(), v8s_t[:])
            nc.sync.dma_start(i8s_o.ap(), i8s_t[:])

            if STOP < 7:
                raise _StopBuild(None)
            # ---------- P6: gather per-partition top-8-by-v8 blocks ----------
            v8col = bass.AP(v8s_t.tensor, v8s_t.offset + 7,
                            [v8s_t.ap[0], [8, NBLK]])
            s8 = spool.tile([128, 8], F32)
            nc.vector.max(s8[:], v8col)
            si8 = spool.tile([128, 8], U32)
            nc.vector.max_index(si8[:], s8[:], v8col)
            nc.sync.dma_start(sblk_o.ap(), si8[:])
            pio = spool.tile([128, 1], U32)
            nc.gpsimd.iota(pio[:], pattern=[[1, 1]], base=0,
                           channel_multiplier=1)
            shl = spool.tile([128, 8], U32)
            nc.vector.tensor_scalar(shl[:], si8[:], 7, None,
                                    op0=mybir.AluOpType.logical_shift_left)
            rowi = spool.tile([128, 8], U32)
            nc.vector.tensor_tensor(out=rowi[:], in0=shl[:],
                                    in1=pio[:].to_broadcast([128, 8]),
                                    op=mybir.AluOpType.add)
            for s in range(8):
                gt = gpool.tile([128, TW], F32)
                if KN("K_SKIP_GATHER") or KN("K_SKIP_SPILL"):
                    nc.vector.memset(gt[:], 0.0)
                else:
                    nc.gpsimd.indirect_dma_start(
                        out=gt[:], out_offset=None,
                        in_=a_dram.ap(),
                        in_offset=bass.IndirectOffsetOnAxis(ap=rowi[:, s:s + 1],
                                                            axis=0))
                nc.sync.dma_start(sdat_o.ap()[:, s * TW:(s + 1) * TW], gt[:])

      except _StopBuild:
        pass
    nc.compile()
    return nc


def _pos_features():
    """Bit-exact match of the reference's jax positional encoding.

    The reference runs on XLA-CPU, whose f32 sin/cos at large arguments
    (up to 2^15) differs from correctly-rounded sin by up to ~8e-4 — enough
    to flip the top-64 selection. Replicate it exactly with jax-on-CPU.
    """
    try:
        import jax
        import jax.numpy as jnp
        with jax.default_device(jax.devices("cpu")[0]):
            p = jnp.linspace(-1.0, 1.0, N)
            feats = [p] + [f((2.0 ** i) * p) for i in range(16)
                           for f in (jnp.sin, jnp.cos)]
            return np.asarray(jnp.stack(feats, 0), np.float32)
    except Exception:
        p = np.linspace(-1.0, 1.0, N, dtype=np.float32)
        feats = [p]
        for i in range(16):
            arg = (np.float32(2.0 ** i) * p).astype(np.float64)
            feats.append(np.sin(arg).astype(np.float32))
            feats.append(np.cos(arg).astype(np.float32))
        return np.stack(feats, 0)  # [33, N]


def kernel(x, bank, atoms, reduce_w, to_atoms_w, dilated_ws, conv_ws,
           _return_results=False, _trace=False):
    x = np.asarray(x, np.float32)
    bank = np.asarray(bank, np.float32)
    atoms = np.asarray(atoms, np.float32)
    reduce_w = np.asarray(reduce_w, np.float32)
    to_atoms_w = np.asarray(to_atoms_w, np.float32)
    dilated_ws = np.asarray(dilated_ws, np.float32)
    conv_ws = np.asarray(conv_ws, np.float32)

    if "prog" not in _CACHE:
        _CACHE["prog"] = _build_program()
    nc = _CACHE["prog"]

    # ---- host-side weight layouts ----
    bankT = np.ascontiguousarray(bank[:, 0, :].T)                  # [512k,128c] -> [128p? ...]
    # bankT tile layout [128, 512]: [p, kc*128 + c] = bank[c, kc*128+p]
    import ml_dtypes
    bf16 = ml_dtypes.bfloat16
    bankT_sb = np.zeros((128, 512), np.float32)
    for kc in range(4):
        bankT_sb[:, kc * 128:(kc + 1) * 128] = bank[:, 0, kc * 128:(kc + 1) * 128].T
    bankT_hi = bankT_sb.astype(bf16)
    bankT_lo = (bankT_sb - bankT_hi.astype(np.float32)).astype(bf16)
    redTh = np.ascontiguousarray(reduce_w[:, :128, 0].T)           # [128ci,128co]
    redTp = np.ascontiguousarray(reduce_w[:, 128:, 0].T)           # [33, 128]
    dilT_sb = np.zeros((128, 18 * 128), np.float32)
    for i in range(6):
        w2 = conv_ws[i, :, :, 0].astype(np.float64)       # [co, c2]
        for tap in range(3):
            comp = w2 @ dilated_ws[i, :, :, tap].astype(np.float64)  # [co, ci]
            dilT_sb[:, (i * 3 + tap) * 128:(i * 3 + tap + 1) * 128] = \
                comp.T.astype(np.float32)
    atomT_sb = np.ascontiguousarray(to_atoms_w[:, :, 0].T)         # [128ci, 512ao]

    pos = _pos_features()                                          # [33, N]

    in_maps = []
    for q in range(N_CORES):
        b, hf = divmod(q, 2)
        t0 = hf * HALF
        v0 = t0 - MARG
        g0 = v0 - GUARD
        # xpad[i] = x[b, v0-512+i], zero outside
        xb = v0 - KS
        xp = np.zeros((1, XPAD_LEN), np.float32)
        lo, hi = max(0, xb), min(N, xb + XPAD_LEN)
        if hi > lo:
            xp[0, lo - xb:hi - xb] = x[b, lo:hi]
        xp_hi = xp.astype(bf16)
        xp_lo = (xp - xp_hi.astype(np.float32)).astype(bf16)
        # posb aligned to buffer cols: col c -> global g0 + c
        pb = np.zeros((33, BW), np.float32)
        lo, hi = max(0, g0), min(N, g0 + BW)
        if hi > lo:
            pb[:, lo - g0:hi - g0] = pos[:, lo:hi]
        # masks for tiles 0 and 32 (buffer cols [128,640) and [16512,17024))
        m0 = np.zeros((128, TW), np.float32)
        m32 = np.zeros((128, TW), np.float32)
        for j in range(TW):
            g_t0 = g0 + GUARD + j
            g_t32 = g0 + GUARD + (NWT - 1) * TW + j
            m0[:, j] = 1.0 if 0 <= g_t0 < N else 0.0
            m32[:, j] = 1.0 if 0 <= g_t32 < N else 0.0
        in_maps.append(dict(
            xpad_h=xp_hi, xpad_l=xp_lo, posb=pb,
            bankT_h=bankT_hi, bankT_l=bankT_lo, redTh=redTh, redTp=redTp,
            dilT=dilT_sb, atomT=atomT_sb,
            mask0=m0, mask32=m32))

    res = run_bass_kernel_spmd(nc, in_maps, list(range(N_CORES)),
                               trace=_trace)
    results = res.results

    # ---- host merge ----
    out = np.zeros((B, 1, N + 2 * ASZ - 1), np.float32)
    context = np.zeros((B, C), np.float32)
    n = N
    for b in range(B):
        cand_vals = []
        cand_flat = []
        for hf in range(2):
            r = results[2 * b + hf]
            v8 = r["v8s"].reshape(128, NBLK, 8)
            i8 = r["i8s"].reshape(128, NBLK, 8).astype(np.int64)
            r["_v8"] = v8
            r["_i8"] = i8
        # t8 = 64th largest of the union of per-block top-8s
        allv = np.concatenate([results[2 * b + hf]["_v8"].ravel()
                               for hf in range(2)])
        t8 = np.partition(allv, -KSP)[-KSP]
        for hf in range(2):
            r = results[2 * b + hf]
            v8, i8 = r["_v8"], r["_i8"]
            sblk = r["sblk"].astype(np.int64)            # [128, 8]
            sdat = r["sdat"].reshape(128, 8, TW)
            susp = v8[:, :, 7] >= t8                     # [128, NBLK]
            ps, bis = np.nonzero(susp)
            # every suspicious block must be among the gathered ones
            gathered = {(p, sblk[p, s]) for p in range(128) for s in range(8)}
            for p, bi in zip(ps.tolist(), bis.tolist()):
                assert (p, bi) in gathered, "suspicious block not gathered"
            base_t = hf * HALF
            # non-suspicious: top-8 values with positions
            nsp = ~susp
            pp, bb = np.nonzero(nsp)
            if pp.size:
                vals = v8[pp, bb, :].ravel()
                idxs = i8[pp, bb, :].ravel()
                ac = bb // NET
                k = bb % NET
                at = ac * 128 + pp
                tau = base_t + k * TW
                flat = ((at[:, None] * n) + tau[:, None] + idxs.reshape(-1, 8)).ravel()
                keep = vals >= t8
                cand_vals.append(vals[keep])
                cand_flat.append(flat[keep])
            # suspicious: all 512 values from gathered data
            slot_of = {}
            for p in range(128):
                for s in range(8):
                    slot_of[(p, sblk[p, s])] = s
            for p, bi in zip(ps.tolist(), bis.tolist()):
                s = slot_of[(p, bi)]
                vals = sdat[p, s, :]
                ac, k = divmod(bi, NET)
                at = ac * 128 + p
                tau = base_t + k * TW
                flat = at * n + tau + np.arange(TW)
                keep = vals >= t8
                cand_vals.append(vals[keep])
                cand_flat.append(flat[keep])
        cand_vals = np.concatenate(cand_vals)
        cand_flat = np.concatenate(cand_flat)
        # dedupe (a block's top-8 can't double-count: susp excluded; but be safe)
        _, uniq = np.unique(cand_flat, return_index=True)
        cand_vals, cand_flat = cand_vals[uniq], cand_flat[uniq]
        # exact top-64, ties by lowest flat index
        order = np.lexsort((cand_flat, -cand_vals))[:KSP]
        for ci in order:
            v = cand_vals[ci]
            at, tau = divmod(int(cand_flat[ci]), n)
            out[b, 0, tau + ASZ:tau + 2 * ASZ] += v * atoms[at]
        context[b] = np.maximum(results[2 * b]["ctxo"][:, 0],
                                results[2 * b + 1]["ctxo"][:, 0])

    if _return_results:
        return (out, context), res
    return out, context


# revision 18
# speedup vs baseline: 1.3205x; 1.0993x over previous
"""Trainium2 Bass kernel for nn_AnalysisBand (topk_masking).

Strategy (8 NeuronCores, SPMD):
  - core q handles example b = q//2, time half hf = q%2 (16384 samples + halo).
  - full forward (filterbank conv, example-norm, pos-enc reduce conv, 6-block
    dilated residual stack, to_atoms conv + relu) runs on-device.
  - matmuls use exact bf16 hi/lo split arithmetic (x = hi + lo, products
    W@A ~= Wh@Ah + Wh@Al + Wl@Ah): bf16xbf16 products are exact in the f32
    PSUM, so the result matches fp32 to ~2^-18 at 1 cycle/row instead of
    fp32's 4 cycles/row. Activations are carried as (hi, lo) bf16 pairs so
    the residual chain keeps f32-level precision.
  - the residual add is folded into the PE via identity-matmul accumulation.
  - the 1x1 conv of every residual block is composed into the dilated conv
    weights on the host (exact linear composition).
  - example-norm statistics are all-reduced over the 2-core pair with one
    tiny collective; the normalization itself is folded into the reduce
    conv's affine epilogue so no matmul waits on the collective.
  - top-k: per (atom, 512-sample) block the device tracks top-8 values +
    indices (DVE max8/max_index), spills the full relu'd activation map to
    device DRAM, and gathers the per-partition top-8-by-v8 blocks (a provable
    superset of any block that can hold >8 of the example's top-64).
  - host merges the per-core block-top-8 lists exactly (threshold t8 = 64th
    largest of the union; blocks with v8 >= t8 are read from the gathered
    full blocks), reconstructs the exact global top-64 (values+positions),
    and scatters the 64 scaled atoms into the output (the final conv of a
    64-nonzero sparse tensor). Context is the max of the two halves' maxima.

Self-contained: only needs the installed concourse/bass toolchain.
"""

import sys
from contextlib import ExitStack

import numpy as np

if "/opt/trn_rl_repo" not in sys.path:
    sys.path.insert(0, "/opt/trn_rl_repo")

import concourse.bass as bass
import concourse.tile as tile
from concourse import bacc, mybir
from concourse.bass_utils import run_bass_kernel_spmd

# ---------------- problem constants (hardcoded) ----------------
B = 4
N = 32768
C = 128
KS = 512            # filterbank kernel size
NA = 512            # n atoms
ASZ = 512           # atom size
KSP = 64            # k sparse
DILS = (1, 3, 9, 27, 81, 1)

N_CORES = 8
HALF = N // 2       # 16384 per core
TW = 512            # time tile width
NWT = 33            # workspace tiles (16896 cols)
NET = 32            # exclusive-region tiles
GUARD = 128         # zero guard columns each side
MARG = 256          # halo margin each side
WS = NWT * TW       # 16896 workspace columns
BW = GUARD + WS + GUARD   # 17152 buffer columns
EXC0 = GUARD + MARG       # buffer col of exclusive-region start (384)
XPAD_LEN = 17408          # x window per core: [v0-512, v0+16896)

NBLK = 4 * NET            # 128 (atom-chunk, tile) blocks per partition row
F32 = mybir.dt.float32
U32 = mybir.dt.uint32
BF16 = mybir.dt.bfloat16

_PAIR_GROUPS = [[0, 1], [2, 3], [4, 5], [6, 7]]

_CACHE = {}


def _build_program():
    nc = bacc.Bacc("TRN2", target_bir_lowering=False, debug=False,
                   num_devices=N_CORES)

    # ---- per-core external inputs ----
    xpad_h = nc.dram_tensor("xpad_h", [1, XPAD_LEN], BF16, kind="ExternalInput")
    xpad_l = nc.dram_tensor("xpad_l", [1, XPAD_LEN], BF16, kind="ExternalInput")
    posb = nc.dram_tensor("posb", [33, BW], F32, kind="ExternalInput")
    bankT_h = nc.dram_tensor("bankT_h", [128, 512], BF16, kind="ExternalInput")
    bankT_l = nc.dram_tensor("bankT_l", [128, 512], BF16, kind="ExternalInput")
    redTh_h = nc.dram_tensor("redTh_h", [128, 128], BF16, kind="ExternalInput")
    redTh_l = nc.dram_tensor("redTh_l", [128, 128], BF16, kind="ExternalInput")
    redTp = nc.dram_tensor("redTp", [33, 128], F32, kind="ExternalInput")
    dilT_h = nc.dram_tensor("dilT_h", [128, 18 * 128], BF16, kind="ExternalInput")
    dilT_l = nc.dram_tensor("dilT_l", [128, 18 * 128], BF16, kind="ExternalInput")
    atomT_h = nc.dram_tensor("atomT_h", [128, 512], BF16, kind="ExternalInput")
    atomT_l = nc.dram_tensor("atomT_l", [128, 512], BF16, kind="ExternalInput")
    ident_b = nc.dram_tensor("ident_b", [128, 128], BF16, kind="ExternalInput")
    mask0 = nc.dram_tensor("mask0", [128, TW], F32, kind="ExternalInput")
    mask32 = nc.dram_tensor("mask32", [128, TW], F32, kind="ExternalInput")

    # ---- outputs ----
    v8s_o = nc.dram_tensor("v8s", [128, NBLK * 8], F32, kind="ExternalOutput")
    i8s_o = nc.dram_tensor("i8s", [128, NBLK * 8], U32, kind="ExternalOutput")
    sblk_o = nc.dram_tensor("sblk", [128, 8], U32, kind="ExternalOutput")
    sdat_o = nc.dram_tensor("sdat", [128, 8 * TW], F32, kind="ExternalOutput")
    ctx_o = nc.dram_tensor("ctxo", [128, 1], F32, kind="ExternalOutput")
    stat_o = nc.dram_tensor("stato", [1, 4], F32, kind="ExternalOutput")

    # ---- internal DRAM ----
    a_dram = nc.dram_tensor("a_dram", [NBLK * 128, TW], F32)   # [bi*128+p, 512]
    st_in = nc.dram_tensor("st_in", [1, 2], F32)
    st_out = nc.dram_tensor("st_out", [1, 2], F32)

    Af = mybir.ActivationFunctionType
    Alu = mybir.AluOpType

    with tile.TileContext(nc) as tc:
        with ExitStack() as ctx:
            wpool = ctx.enter_context(tc.tile_pool(name="w", bufs=1))
            hpool = ctx.enter_context(tc.tile_pool(name="hp", bufs=1))
            impool = ctx.enter_context(tc.tile_pool(name="im", bufs=3))
            pspool = ctx.enter_context(
                tc.tile_pool(name="ps", bufs=3, space="PSUM"))
            tpool = ctx.enter_context(tc.tile_pool(name="tp", bufs=2))
            apool = ctx.enter_context(tc.tile_pool(name="ap", bufs=3))
            spool = ctx.enter_context(tc.tile_pool(name="st", bufs=1))
            pppool = ctx.enter_context(tc.tile_pool(name="pp", bufs=3))
            gpool = ctx.enter_context(tc.tile_pool(name="gp", bufs=2))

            # ---- early weight loads (needed in P1/P2) ----
            bank_h = wpool.tile([128, 512], BF16)
            nc.sync.dma_start(bank_h[:], bankT_h.ap())
            bank_l = wpool.tile([128, 512], BF16)
            nc.sync.dma_start(bank_l[:], bankT_l.ap())
            redh_h = wpool.tile([128, 128], BF16)
            nc.sync.dma_start(redh_h[:], redTh_h.ap())
            redh_l = wpool.tile([128, 128], BF16)
            nc.sync.dma_start(redh_l[:], redTh_l.ap())
            redp_t = wpool.tile([33, 128], F32)
            nc.sync.dma_start(redp_t[:], redTp.ap())

            # activation buffers: two ping-pong (hi, lo) bf16 pairs
            hAh = hpool.tile([128, BW], BF16)
            hAl = hpool.tile([128, BW], BF16)
            hBh = hpool.tile([128, BW], BF16)
            hBl = hpool.tile([128, BW], BF16)
            for buf in (hAh, hAl, hBh, hBl):
                nc.vector.memset(buf[:, 0:GUARD], 0.0)
                nc.vector.memset(buf[:, GUARD + WS:BW], 0.0)

            # stats accumulators
            ssum = spool.tile([128, NWT], F32)
            ssq = spool.tile([128, NWT], F32)

            # ---------- P1: filterbank conv (bf16 split) + stats ----------
            for k in range(NWT):
                xim_h = impool.tile([128, 896], BF16, tag="xh")
                xim_l = impool.tile([128, 896], BF16, tag="xl")
                if k == 0:
                    for q4 in range(4):
                        nc.sync.dma_start(
                            xim_h[q4 * 32:(q4 + 1) * 32, :],
                            bass.AP(xpad_h, q4 * 32, [[1, 32], [1, 896]]))
                        nc.sync.dma_start(
                            xim_l[q4 * 32:(q4 + 1) * 32, :],
                            bass.AP(xpad_l, q4 * 32, [[1, 32], [1, 896]]))
                else:
                    nc.sync.dma_start(xim_h[:],
                                      bass.AP(xpad_h, TW * k, [[1, 128], [1, 896]]))
                    nc.sync.dma_start(xim_l[:],
                                      bass.AP(xpad_l, TW * k, [[1, 128], [1, 896]]))
                ps = pspool.tile([128, TW], F32)
                imm, nmm = 0, 12
                for kc in range(4):
                    for (bw, xw) in ((bank_h, xim_h), (bank_h, xim_l),
                                     (bank_l, xim_h)):
                        nc.tensor.matmul(
                            ps[:], bw[:, kc * 128:(kc + 1) * 128],
                            xw[:, kc * 128:kc * 128 + TW],
                            start=(imm == 0), stop=(imm == nmm - 1))
                        imm += 1
                cc = slice(GUARD + k * TW, GUARD + (k + 1) * TW)
                nc.scalar.copy(hAh[:, cc], ps[:])
                nc.vector.tensor_tensor(out=hAl[:, cc], in0=ps[:],
                                        in1=hAh[:, cc], op=Alu.subtract)
                # stats over the exclusive region only (from full-precision PSUM)
                if k == 0:
                    sub = ps[:, MARG:TW]
                elif k == NWT - 1:
                    sub = ps[:, 0:MARG]
                else:
                    sub = ps[:]
                nc.vector.reduce_sum(ssum[:, k:k + 1], sub,
                                     axis=mybir.AxisListType.X)
                scr = tpool.tile([128, TW], F32, tag="z")
                nc.scalar.activation(scr[:, 0:sub.shape[-1]], sub, Af.Square,
                                     accum_out=ssq[:, k:k + 1])

            # late weight loads (needed from P3/P5 on)
            dil_h = wpool.tile([128, 18 * 128], BF16)
            nc.sync.dma_start(dil_h[:], dilT_h.ap())
            dil_l = wpool.tile([128, 18 * 128], BF16)
            nc.sync.dma_start(dil_l[:], dilT_l.ap())
            atom_h = wpool.tile([128, 512], BF16)
            nc.sync.dma_start(atom_h[:], atomT_h.ap())
            atom_l = wpool.tile([128, 512], BF16)
            nc.sync.dma_start(atom_l[:], atomT_l.ap())
            id_b = wpool.tile([128, 128], BF16)
            nc.sync.dma_start(id_b[:], ident_b.ap())
            m0_t = wpool.tile([128, TW], F32)
            nc.sync.dma_start(m0_t[:], mask0.ap())
            m32_t = wpool.tile([128, TW], F32)
            nc.sync.dma_start(m32_t[:], mask32.ap())

            # ---------- stats all-reduce over the pair ----------
            s1 = spool.tile([128, 1], F32)
            s2 = spool.tile([128, 1], F32)
            nc.vector.reduce_sum(s1[:], ssum[:], axis=mybir.AxisListType.X)
            nc.vector.reduce_sum(s2[:], ssq[:], axis=mybir.AxisListType.X)
            ones128 = spool.tile([128, 1], F32)
            nc.vector.memset(ones128[:], 1.0)
            ones1w = spool.tile([1, 128], F32)
            nc.vector.memset(ones1w[:], 1.0)
            stin = spool.tile([1, 2], F32)
            psA = pspool.tile([1, 1], F32, tag="ps2", bufs=1)
            nc.tensor.matmul(psA[:], s1[:], ones128[:], start=True, stop=True)
            nc.vector.tensor_copy(stin[0:1, 0:1], psA[:])
            psB = pspool.tile([1, 1], F32, tag="ps2", bufs=1)
            nc.tensor.matmul(psB[:], s2[:], ones128[:], start=True, stop=True)
            nc.vector.tensor_copy(stin[0:1, 1:2], psB[:])
            nc.sync.dma_start(st_in.ap(), stin[:])
            nc.gpsimd.collective_compute(
                "AllReduce", Alu.add,
                replica_groups=_PAIR_GROUPS,
                ins=[st_in.ap()], outs=[st_out.ap()])
            stout = spool.tile([1, 2], F32)
            nc.sync.dma_start(stout[:], st_out.ap())
            # mu = S/(C*N); m2 = Q/(C*N); var = m2 - mu^2
            invN = 1.0 / (C * N)
            mu = spool.tile([1, 1], F32)
            nc.scalar.mul(mu[:], stout[0:1, 0:1], invN)
            m2 = spool.tile([1, 1], F32)
            nc.scalar.mul(m2[:], stout[0:1, 1:2], invN)
            musq = spool.tile([1, 1], F32)
            nc.vector.tensor_tensor(out=musq[:], in0=mu[:], in1=mu[:],
                                    op=Alu.mult)
            var = spool.tile([1, 1], F32)
            nc.vector.tensor_tensor(out=var[:], in0=m2[:], in1=musq[:],
                                    op=Alu.subtract)
            sig = spool.tile([1, 1], F32)
            nc.scalar.sqrt(sig[:], var[:])
            sige = spool.tile([1, 1], F32)
            nc.vector.tensor_scalar_add(sige[:], sig[:], 1e-8)
            inv = spool.tile([1, 1], F32)
            nc.vector.reciprocal(inv[:], sige[:])
            nbias = spool.tile([1, 1], F32)
            nc.vector.tensor_tensor(out=nbias[:], in0=mu[:], in1=inv[:],
                                    op=Alu.mult)
            nc.scalar.mul(nbias[:], nbias[:], -1.0)
            invb = spool.tile([128, 1], F32)
            psC = pspool.tile([128, 1], F32, tag="ps2", bufs=1)
            nc.tensor.matmul(psC[:], ones1w[:], inv[:], start=True, stop=True)
            nc.vector.tensor_copy(invb[:], psC[:])
            nbb = spool.tile([128, 1], F32)
            psD = pspool.tile([128, 1], F32, tag="ps2", bufs=1)
            nc.tensor.matmul(psD[:], ones1w[:], nbias[:], start=True, stop=True)
            nc.vector.tensor_copy(nbb[:], psD[:])
            # c1[co] = nbias * sum_ci redh[ci, co]  (rowsums via PE, fp32 W)
            redh_f = wpool.tile([128, 128], F32)
            nc.vector.tensor_tensor(out=redh_f[:], in0=redh_h[:],
                                    in1=redh_l[:], op=Alu.add)
            psRS = pspool.tile([128, 1], F32, tag="ps2", bufs=1)
            nc.tensor.matmul(psRS[:], redh_f[:], ones128[:], start=True,
                             stop=True)
            c1b = spool.tile([128, 1], F32)
            nc.scalar.activation(c1b[:], psRS[:], Af.Copy, scale=nbb[:])
            stdbg = spool.tile([1, 4], F32)
            nc.vector.tensor_copy(stdbg[0:1, 0:1], mu[:])
            nc.vector.tensor_copy(stdbg[0:1, 1:2], sig[:])
            nc.vector.tensor_copy(stdbg[0:1, 2:3], stout[0:1, 0:1])
            nc.vector.tensor_copy(stdbg[0:1, 3:4], stout[0:1, 1:2])
            nc.sync.dma_start(stat_o.ap(), stdbg[:])

            # ---------- P2: reduce conv (161 -> 128), norm folded ----------
            # y = inv*(Wh @ h_raw) + c1 + Wp @ pos == Wh @ ((h-mu)/sig) + Wp @ pos
            for k in range(NWT):
                cc = slice(GUARD + k * TW, GUARD + (k + 1) * TW)
                pp = pppool.tile([33, TW], F32)
                nc.sync.dma_start(pp[:], posb.ap()[:, cc])
                ps = pspool.tile([128, TW], F32)
                imm = 0
                for (ww, aa) in ((redh_h, hAh), (redh_h, hAl), (redh_l, hAh)):
                    nc.tensor.matmul(ps[:], ww[:], aa[:, cc],
                                     start=(imm == 0), stop=(imm == 2))
                    imm += 1
                psP = pspool.tile([128, TW], F32, tag="ps1")
                nc.tensor.matmul(psP[:], redp_t[:], pp[:],
                                 start=True, stop=True)
                t2 = tpool.tile([128, TW], F32, tag="u")
                nc.scalar.activation(t2[:], ps[:], Af.Identity,
                                     bias=c1b[:], scale=invb[:])
                yf = tpool.tile([128, TW], F32, tag="z")
                nc.vector.tensor_tensor(out=yf[:], in0=t2[:], in1=psP[:],
                                        op=Alu.add)
                if k == 0 or k == NWT - 1:
                    mt = m0_t if k == 0 else m32_t
                    ym = tpool.tile([128, TW], F32, tag="u2")
                    nc.vector.tensor_tensor(out=ym[:], in0=yf[:], in1=mt[:],
                                            op=Alu.mult)
                    yf = ym
                nc.scalar.copy(hBh[:, cc], yf[:])
                nc.vector.tensor_tensor(out=hBl[:, cc], in0=yf[:],
                                        in1=hBh[:, cc], op=Alu.subtract)

            # ---------- P3: dilated residual stack (bf16 split, residual
            # folded into the PE, 1x1 conv composed into the taps) ----------
            pairs = ((hBh, hBl), (hAh, hAl))
            ctxcol = spool.tile([128, NWT], F32)
            for i, d in enumerate(DILS):
                INh, INl = pairs[i % 2]
                OUTh, OUTl = pairs[(i + 1) % 2]
                last = (i == len(DILS) - 1)
                for k in range(NWT):
                    c0 = GUARD + k * TW
                    cc = slice(c0, c0 + TW)
                    ps1 = pspool.tile([128, TW], F32, tag="ps1")
                    imm = 0
                    for tap in range(3):
                        off = (tap - 1) * d
                        wslice = slice((i * 3 + tap) * 128,
                                       (i * 3 + tap + 1) * 128)
                        vs = slice(c0 + off, c0 + off + TW)
                        for (ww, aa) in ((dil_h, INh), (dil_h, INl),
                                         (dil_l, INh)):
                            nc.tensor.matmul(ps1[:], ww[:, wslice], aa[:, vs],
                                             start=(imm == 0), stop=False)
                            imm += 1
                    # residual: += IN (hi and lo) via identity matmul
                    nc.tensor.matmul(ps1[:], id_b[:], INh[:, cc],
                                     start=False, stop=False)
                    nc.tensor.matmul(ps1[:], id_b[:], INl[:, cc],
                                     start=False, stop=True)
                    if k == 0 or k == NWT - 1:
                        mt = m0_t if k == 0 else m32_t
                        zi = tpool.tile([128, TW], F32, tag="z")
                        nc.vector.tensor_tensor(out=zi[:], in0=ps1[:],
                                                in1=mt[:], op=Alu.mult)
                        src_u = zi
                    else:
                        src_u = ps1
                    # leaky relu: max(u, 0.2u)
                    ul = tpool.tile([128, TW], F32, tag="ul")
                    nc.scalar.mul(ul[:], src_u[:], 0.2)
                    uf = tpool.tile([128, TW], F32, tag="u")
                    nc.vector.tensor_tensor(out=uf[:], in0=src_u[:],
                                            in1=ul[:], op=Alu.max)
                    nc.scalar.copy(OUTh[:, cc], uf[:])
                    nc.vector.tensor_tensor(out=OUTl[:, cc], in0=uf[:],
                                            in1=OUTh[:, cc], op=Alu.subtract)
                    if last:
                        # context partial maxima over the exclusive region
                        if k == 0:
                            subf = uf[:, MARG:TW]
                        elif k == NWT - 1:
                            subf = uf[:, 0:MARG]
                        else:
                            subf = uf[:]
                        nc.vector.reduce_max(ctxcol[:, k:k + 1], subf,
                                             axis=mybir.AxisListType.X)

            FEATh, FEATl = pairs[0]  # 6 blocks: ends back in (hBh, hBl)

            # ---------- P4: context ----------
            ctx_t = spool.tile([128, 1], F32)
            nc.vector.reduce_max(ctx_t[:], ctxcol[:], axis=mybir.AxisListType.X)
            nc.sync.dma_start(ctx_o.ap(), ctx_t[:])

            # ---------- P5: to_atoms + relu + block top-8 + spill ----------
            v8s_t = spool.tile([128, NBLK * 8], F32)
            i8s_t = spool.tile([128, NBLK * 8], U32)
            for k in range(NET):
                c0 = EXC0 + k * TW
                cc = slice(c0, c0 + TW)
                for ac in range(4):
                    ws_ = slice(ac * 128, (ac + 1) * 128)
                    ps = pspool.tile([128, TW], F32, tag="ps")
                    imm = 0
                    for (ww, aa) in ((atom_h, FEATh), (atom_h, FEATl),
                                     (atom_l, FEATh)):
                        nc.tensor.matmul(ps[:], ww[:, ws_], aa[:, cc],
                                         start=(imm == 0), stop=(imm == 2))
                        imm += 1
                    at = apool.tile([128, TW], F32)
                    nc.scalar.activation(at[:], ps[:], Af.Relu)
                    bi = ac * NET + k
                    nc.vector.max(v8s_t[:, bi * 8:(bi + 1) * 8], at[:])
                    nc.vector.max_index(i8s_t[:, bi * 8:(bi + 1) * 8],
                                        v8s_t[:, bi * 8:(bi + 1) * 8], at[:])
                    nc.sync.dma_start(
                        a_dram.ap()[bi * 128:(bi + 1) * 128, :], at[:])
            nc.sync.dma_start(v8s_o.ap(), v8s_t[:])
            nc.sync.dma_start(i8s_o.ap(), i8s_t[:])

            # ---------- P6: gather per-partition top-8-by-v8 blocks ----------
            v8col = bass.AP(v8s_t.tensor, v8s_t.offset + 7,
                            [v8s_t.ap[0], [8, NBLK]])
            s8 = spool.tile([128, 8], F32)
            nc.vector.max(s8[:], v8col)
            si8 = spool.tile([128, 8], U32)
            nc.vector.max_index(si8[:], s8[:], v8col)
            nc.sync.dma_start(sblk_o.ap(), si8[:])
            pio = spool.tile([128, 1], U32)
            nc.gpsimd.iota(pio[:], pattern=[[1, 1]], base=0,
                           channel_multiplier=1)
            shl = spool.tile([128, 8], U32)
            nc.vector.tensor_scalar(shl[:], si8[:], 7, None,
                                    op0=Alu.logical_shift_left)
            rowi = spool.tile([128, 8], U32)
            nc.vector.tensor_tensor(out=rowi[:], in0=shl[:],
                                    in1=pio[:].to_broadcast([128, 8]),
                                    op=Alu.add)
            for s in range(8):
                gt = gpool.tile([128, TW], F32)
                nc.gpsimd.indirect_dma_start(
                    out=gt[:], out_offset=None,
                    in_=a_dram.ap(),
                    in_offset=bass.IndirectOffsetOnAxis(ap=rowi[:, s:s + 1],
                                                        axis=0))
                nc.sync.dma_start(sdat_o.ap()[:, s * TW:(s + 1) * TW], gt[:])

    nc.compile()
    return nc


def _pos_features():
    """Bit-exact match of the reference's jax positional encoding.

    The reference runs on XLA-CPU, whose f32 sin/cos at large arguments
    (up to 2^15) differs from correctly-rounded sin by up to ~8e-4 — enough
    to flip the top-64 selection. Replicate it exactly with jax-on-CPU.
    """
    try:
        import jax
        import jax.numpy as jnp
        with jax.default_device(jax.devices("cpu")[0]):
            p = jnp.linspace(-1.0, 1.0, N)
            feats = [p] + [f((2.0 ** i) * p) for i in range(16)
                           for f in (jnp.sin, jnp.cos)]
            return np.asarray(jnp.stack(feats, 0), np.float32)
    except Exception:
        p = np.linspace(-1.0, 1.0, N, dtype=np.float32)
        feats = [p]
        for i in range(16):
            arg = (np.float32(2.0 ** i) * p).astype(np.float64)
            feats.append(np.sin(arg).astype(np.float32))
            feats.append(np.cos(arg).astype(np.float32))
        return np.stack(feats, 0)  # [33, N]


def _split_bf16(a):
    import ml_dtypes
    bf16 = ml_dtypes.bfloat16
    hi = np.ascontiguousarray(a, np.float32).astype(bf16)
    lo = (a - hi.astype(np.float32)).astype(bf16)
    return hi, lo


def kernel(x, bank, atoms, reduce_w, to_atoms_w, dilated_ws, conv_ws,
           _return_results=False, _trace=False):
    import ml_dtypes
    bf16 = ml_dtypes.bfloat16
    x = np.asarray(x, np.float32)
    bank = np.asarray(bank, np.float32)
    atoms = np.asarray(atoms, np.float32)
    reduce_w = np.asarray(reduce_w, np.float32)
    to_atoms_w = np.asarray(to_atoms_w, np.float32)
    dilated_ws = np.asarray(dilated_ws, np.float32)
    conv_ws = np.asarray(conv_ws, np.float32)

    if "prog" not in _CACHE:
        _CACHE["prog"] = _build_program()
    nc = _CACHE["prog"]

    # ---- host-side weight layouts ----
    bankT_sb = np.zeros((128, 512), np.float32)
    for kc in range(4):
        bankT_sb[:, kc * 128:(kc + 1) * 128] = \
            bank[:, 0, kc * 128:(kc + 1) * 128].T
    bankT_hi, bankT_lo = _split_bf16(bankT_sb)
    redTh = np.ascontiguousarray(reduce_w[:, :128, 0].T)           # [ci, co]
    redTh_hi, redTh_lo = _split_bf16(redTh)
    redTp = np.ascontiguousarray(reduce_w[:, 128:, 0].T)           # [33, co]
    dilT_sb = np.zeros((128, 18 * 128), np.float32)
    for i in range(6):
        w2 = conv_ws[i, :, :, 0].astype(np.float64)                # [co, c2]
        for tap in range(3):
            comp = w2 @ dilated_ws[i, :, :, tap].astype(np.float64)
            dilT_sb[:, (i * 3 + tap) * 128:(i * 3 + tap + 1) * 128] = \
                comp.T.astype(np.float32)
    dilT_hi, dilT_lo = _split_bf16(dilT_sb)
    atomT_sb = np.ascontiguousarray(to_atoms_w[:, :, 0].T)         # [ci, ao]
    atomT_hi, atomT_lo = _split_bf16(atomT_sb)
    ident = np.eye(128, dtype=np.float32).astype(bf16)

    pos = _pos_features()                                          # [33, N]

    in_maps = []
    for q in range(N_CORES):
        b, hf = divmod(q, 2)
        t0 = hf * HALF
        v0 = t0 - MARG
        g0 = v0 - GUARD
        # xpad[i] = x[b, v0-512+i], zero outside
        xb = v0 - KS
        xp = np.zeros((1, XPAD_LEN), np.float32)
        lo_, hi_ = max(0, xb), min(N, xb + XPAD_LEN)
        if hi_ > lo_:
            xp[0, lo_ - xb:hi_ - xb] = x[b, lo_:hi_]
        xp_hi, xp_lo = _split_bf16(xp)
        # posb aligned to buffer cols: col c -> global g0 + c
        pb = np.zeros((33, BW), np.float32)
        lo_, hi_ = max(0, g0), min(N, g0 + BW)
        if hi_ > lo_:
            pb[:, lo_ - g0:hi_ - g0] = pos[:, lo_:hi_]
        # masks for tiles 0 and 32 (buffer cols [128,640) and [16512,17024))
        m0 = np.zeros((128, TW), np.float32)
        m32 = np.zeros((128, TW), np.float32)
        for j in range(TW):
            g_t0 = g0 + GUARD + j
            g_t32 = g0 + GUARD + (NWT - 1) * TW + j
            m0[:, j] = 1.0 if 0 <= g_t0 < N else 0.0
            m32[:, j] = 1.0 if 0 <= g_t32 < N else 0.0
        in_maps.append(dict(
            xpad_h=xp_hi, xpad_l=xp_lo, posb=pb,
            bankT_h=bankT_hi, bankT_l=bankT_lo,
            redTh_h=redTh_hi, redTh_l=redTh_lo, redTp=redTp,
            dilT_h=dilT_hi, dilT_l=dilT_lo,
            atomT_h=atomT_hi, atomT_l=atomT_lo,
            ident_b=ident, mask0=m0, mask32=m32))

    res = run_bass_kernel_spmd(nc, in_maps, list(range(N_CORES)),
                               trace=_trace)
    results = res.results

    # ---- host merge ----
    out = np.zeros((B, 1, N + 2 * ASZ - 1), np.float32)
    context = np.zeros((B, C), np.float32)
    n = N
    for b in range(B):
        cand_vals = []
        cand_flat = []
        for hf in range(2):
            r = results[2 * b + hf]
            r["_v8"] = r["v8s"].reshape(128, NBLK, 8)
            r["_i8"] = r["i8s"].reshape(128, NBLK, 8).astype(np.int64)
        # t8 = 64th largest of the union of per-block top-8s
        allv = np.concatenate([results[2 * b + hf]["_v8"].ravel()
                               for hf in range(2)])
        t8 = np.partition(allv, -KSP)[-KSP]
        for hf in range(2):
            r = results[2 * b + hf]
            v8, i8 = r["_v8"], r["_i8"]
            sblk = r["sblk"].astype(np.int64)            # [128, 8]
            sdat = r["sdat"].reshape(128, 8, TW)
            susp = v8[:, :, 7] >= t8                     # [128, NBLK]
            ps_, bis = np.nonzero(susp)
            gathered = {(p, sblk[p, s]) for p in range(128) for s in range(8)}
            for p, bi in zip(ps_.tolist(), bis.tolist()):
                assert (p, bi) in gathered, "suspicious block not gathered"
            base_t = hf * HALF
            # non-suspicious blocks: top-8 values with positions
            nsp = ~susp
            pp_, bb = np.nonzero(nsp)
            if pp_.size:
                vals = v8[pp_, bb, :].ravel()
                idxs = i8[pp_, bb, :]
                ac = bb // NET
                kk = bb % NET
                at = ac * 128 + pp_
                tau = base_t + kk * TW
                flat = ((at[:, None] * n) + tau[:, None] + idxs).ravel()
                keep = vals >= t8
                cand_vals.append(vals[keep])
                cand_flat.append(flat[keep])
            # suspicious blocks: all 512 values from the gathered data
            slot_of = {}
            for p in range(128):
                for s in range(8):
                    slot_of[(p, sblk[p, s])] = s
            for p, bi in zip(ps_.tolist(), bis.tolist()):
                s = slot_of[(p, bi)]
                vals = sdat[p, s, :]
                ac, kk = divmod(bi, NET)
                at = ac * 128 + p
                tau = base_t + kk * TW
                flat = at * n + tau + np.arange(TW)
                keep = vals >= t8
                cand_vals.append(vals[keep])
                cand_flat.append(flat[keep])
        cand_vals = np.concatenate(cand_vals)
        cand_flat = np.concatenate(cand_flat)
        _, uniq = np.unique(cand_flat, return_index=True)
        cand_vals, cand_flat = cand_vals[uniq], cand_flat[uniq]
        # exact top-64, ties broken by lowest flat index
        order = np.lexsort((cand_flat, -cand_vals))[:KSP]
        for ci in order:
            v = cand_vals[ci]
            at, tau = divmod(int(cand_flat[ci]), n)
            out[b, 0, tau + ASZ:tau + 2 * ASZ] += v * atoms[at]
        context[b] = np.maximum(results[2 * b]["ctxo"][:, 0],
                                results[2 * b + 1]["ctxo"][:, 0])

    if _return_results:
        return (out, context), res
    return out, context
